# revision 16
# baseline (speedup 1.0000x reference)
"""Trainium2 Bass kernel for nn_CAFF_3100966388292.

Dual-stream (SAR/OPT) cross-attention fusion net:
  theta/phi/g 1x1-conv projections on both streams, per-sample NxN attention
  maps fused elementwise, both value streams attended, product taken, output
  1x1-conv + residual + channel-mean pool + linear head.

Strategy (pure data parallel, 4 samples per core on 8 cores):
  * Layouts chosen so no on-device transposes of big tensors are needed:
      - theta/phi in (CI, N) layout  (lhsT = host-pretransposed weights)
      - g directly in (N, CI) layout (lhsT = input tile, rhs = w^T)
      - attention logits computed TRANSPOSED: L^T(m,n) = phi^T theta, so the
        contracted dim (m) of att@g lands on PSUM partitions naturally.
  * Softmax denominators via ones-column matmuls on the tensor engine
    (partition-dim sums), applied as a scalar fixup on the pooled row:
      (att@g_x * att@g_y)(n,:) = Ux(:,n)*Uy(:,n) / (Zx(n)*Zy(n))^2
    with U the unnormalized attended values (global EXP_SHIFT cancels too).
  * The final W-projection + residual + channel-mean + head collapse
    algebraically:
      pooled(n) = R2(n)*qraw(n) + (ga/C)*sum(W_b) + rs(n),
      rs(n)     = (go/C)*colsum(opt)(n) + (gs/C)*colsum(sar)(n)  [one PSUM acc]
      qraw(n)   = sum_ci wbar(ci) * Ux(ci,n) * Uy(ci,n),
      wbar      = (ga/C) * W_w.sum(0)
    which removes the (C,CI)x(CI,N) W matmul entirely.
  * bf16 on all matmul operands (host-casts + host-packs inputs/weights into
    partition-major layout for single large contiguous-line DMAs). PSUM and
    the pooled fixup chain stay fp32. Final rel err ~3e-3, dominated by bf16
    rounding of the residual colsum path.
  * The per-sample fixup chain + pooled-row transposes are emitted deferred
    (inside the next sample's projection phase) so the PE never stalls on the
    serial DVE row chain.
"""

import sys
import types

import ml_dtypes
import numpy as np

# The agent image's antenv package lacks axon_hooks; register the equivalent
# NTFF hook so run_bass_kernel_spmd(trace=True) works if ever requested.
try:  # pragma: no cover
    import antenv.axon_hooks  # noqa: F401
except ImportError:
    try:
        from trn_agent_boot.trn_boot import _ntff_profile_via_ctypes

        _hook = _ntff_profile_via_ctypes("/opt/axon/libaxon_pjrt.so")
        _mod = types.ModuleType("antenv.axon_hooks")
        _mod.get_axon_ntff_profile_hook = lambda: _hook
        _mod.set_axon_ntff_profile_hook = lambda h: None
        sys.modules["antenv.axon_hooks"] = _mod
    except Exception:
        pass

import concourse.bass as bass
import concourse.tile as tile
from concourse import bacc, mybir
from concourse.bass_utils import run_bass_kernel_spmd

F32 = mybir.dt.float32
BF16 = mybir.dt.bfloat16
EXP_SHIFT = -12.0  # constant logit shift before exp; cancels exactly in the math

B, C, CI, N, HOUT = 32, 512, 256, 768, 256
NCORES = 8
BPC = B // NCORES  # samples per core
KC = C // 128  # 4 k-chunks over channels
MC = N // 128  # 6 chunks over positions
CIC = CI // 128  # 2 chunks over inner channels
# free-dim split of N into PSUM-bank-legal matmul halves
NH = ((0, 512), (512, 256))

_cached = {}


def _pack(a):
    """(R, F) host array -> (128, R//128 * F) partition-major bf16."""
    a = np.asarray(a, dtype=np.float32)
    r, f = a.shape
    k = r // 128
    return np.ascontiguousarray(
        a.reshape(k, 128, f).transpose(1, 0, 2).reshape(128, k * f)
    ).astype(ml_dtypes.bfloat16)


def _build(has_gb_x, has_gb_y, has_hb):
    nc = bacc.Bacc("TRN2", target_bir_lowering=False, debug=False)
    AF = mybir.ActivationFunctionType

    def mm(out, lhsT, rhs, start, stop):
        nc.tensor.matmul(out, lhsT, rhs, start=start, stop=stop)

    # inputs host-packed to (BPC, 128, KC*N) partition-major bf16
    d_sar = nc.dram_tensor("sar", [BPC, 128, KC * N], BF16, kind="ExternalInput")
    d_opt = nc.dram_tensor("opt", [BPC, 128, KC * N], BF16, kind="ExternalInput")
    # host-pretransposed + packed projection weights, (128, KC*CI) bf16
    d_w = {}
    for nm in ("wt_tx", "wt_px", "wt_ty", "wt_py"):
        d_w[nm] = nc.dram_tensor(nm, [128, KC * CI], BF16, kind="ExternalInput")
    for nm in ("wt_gx", "wt_gy"):  # g weights carry a gamma/C ones column
        d_w[nm] = nc.dram_tensor(nm, [128, KC * (CI + 1)], BF16,
                                 kind="ExternalInput")
    d_hwT = nc.dram_tensor("hwT", [128, MC * HOUT], BF16, kind="ExternalInput")
    d_wbar = nc.dram_tensor("wbar", [CI], BF16, kind="ExternalInput")
    d_tb = {  # theta/phi per-partition bias columns (CI,), fp32 (ACT bias)
        nm: nc.dram_tensor(nm, [CI], F32, kind="ExternalInput")
        for nm in ("b_tx", "b_px", "b_ty", "b_py")
    }
    d_ones = nc.dram_tensor("ones_col", [128, 1], BF16, kind="ExternalInput")
    need_onesr = has_gb_x or has_gb_y or has_hb
    if need_onesr:
        d_onesr = nc.dram_tensor("ones_row", [1, 128], BF16, kind="ExternalInput")
    d_ident = nc.dram_tensor("ident", [4, 4], F32, kind="ExternalInput")
    d_expb = nc.dram_tensor("expb", [128, 1], F32, kind="ExternalInput")
    d_gb = {}
    if has_gb_x:
        d_gb["x"] = nc.dram_tensor("gb_x", [1, CI], BF16, kind="ExternalInput")
    if has_gb_y:
        d_gb["y"] = nc.dram_tensor("gb_y", [1, CI], BF16, kind="ExternalInput")
    if has_hb:
        d_hb = nc.dram_tensor("hb", [1, HOUT], BF16, kind="ExternalInput")
    d_out = nc.dram_tensor("out", [BPC, HOUT], F32, kind="ExternalOutput")

    with tile.TileContext(nc) as tc, \
            tc.tile_pool(name="wts", bufs=1) as wts, \
            tc.tile_pool(name="inp", bufs=2) as inp, \
            tc.tile_pool(name="proj", bufs=1) as proj, \
            tc.tile_pool(name="att", bufs=1) as attp, \
            tc.tile_pool(name="rows", bufs=1) as rows, \
            tc.tile_pool(name="rtmp", bufs=4) as rtmp, \
            tc.tile_pool(name="ps", bufs=4, space="PSUM") as ps:

        # ---- DMAs in strict first-use order: the queues are FIFO, so
        # everything emitted ahead of the first matmul's dependencies delays
        # kernel start ----
        def load_w(nm, cols=CI):
            t = wts.tile([128, KC, cols], BF16, tag=nm, name=nm)
            nc.sync.dma_start(t[:], d_w[nm].ap().rearrange("p (k f) -> p k f", k=KC))
            return t

        w_sb = {}
        w_sb["wt_tx"] = load_w("wt_tx")
        x0 = inp.tile([128, KC, N], BF16, tag="x_t", name="x_t")
        for k in range(KC):
            nc.sync.dma_start(x0[:, k, :], d_sar[0][:, k * N:(k + 1) * N])
        w_sb["wt_px"] = load_w("wt_px")
        w_sb["wt_gx"] = load_w("wt_gx", CI + 1)
        tb_sb = {}
        for nm, d in d_tb.items():
            t = wts.tile([128, CIC], F32, tag=nm, name=nm)
            nc.sync.dma_start(t[:], d.ap().rearrange("(k p) -> p k", p=128))
            tb_sb[nm] = t
        w_sb["wt_ty"] = load_w("wt_ty")
        w_sb["wt_py"] = load_w("wt_py")
        w_sb["wt_gy"] = load_w("wt_gy", CI + 1)
        y0 = inp.tile([128, KC, N], BF16, tag="y_t", name="y_t")
        for k in range(KC):
            nc.sync.dma_start(y0[:, k, :], d_opt[0][:, k * N:(k + 1) * N])

        def load_inputs(s):
            x_t = inp.tile([128, KC, N], BF16, tag="x_t", name="x_t")
            y_t = inp.tile([128, KC, N], BF16, tag="y_t", name="y_t")
            for k in range(KC):
                nc.sync.dma_start(x_t[:, k, :], d_sar[s][:, k * N:(k + 1) * N])
            for k in range(KC):
                nc.sync.dma_start(y_t[:, k, :], d_opt[s][:, k * N:(k + 1) * N])
            return x_t, y_t

        in_tiles = [(x0, y0)]

        # ---- small constants (all needed later than the projections) ----
        wbar = wts.tile([128, CIC], BF16, tag="wbar", name="wbar")
        nc.sync.dma_start(wbar[:], d_wbar.ap().rearrange("(k p) -> p k", p=128))
        ones_col = wts.tile([128, 1], BF16, tag="ones_col", name="ones_col")
        nc.sync.dma_start(ones_col[:], d_ones.ap())
        ident = wts.tile([4, 4], F32, tag="ident", name="ident")
        nc.sync.dma_start(ident[:], d_ident.ap())
        expb = wts.tile([128, 1], F32, tag="expb", name="expb")
        nc.sync.dma_start(expb[:], d_expb.ap())
        hwT = wts.tile([128, MC, HOUT], BF16, tag="hwT", name="hwT")
        nc.sync.dma_start(hwT[:], d_hwT.ap().rearrange("p (k f) -> p k f", k=MC))
        if need_onesr:
            ones_row = wts.tile([1, 128], BF16, tag="ones_row", name="ones_row")
            nc.sync.dma_start(ones_row[:], d_onesr.ap())
        gb_sb = {}
        for st, d in d_gb.items():
            t = wts.tile([1, CI], BF16, tag=f"gb_{st}", name=f"gb_{st}")
            nc.sync.dma_start(t[:], d.ap())
            gb_sb[st] = t
        if has_hb:
            hb = wts.tile([1, HOUT], BF16, tag="hb", name="hb")
            nc.sync.dma_start(hb[:], d_hb.ap())

        pooledT = rows.tile([128, MC, BPC], BF16, tag="pooledT", name="pooledT")

        def emit_fixup_tail(fx):
            """qraw matvec, chain B, transposes + colsum-column adds."""
            s, p3, yv, wbar_, rscol = fx
            pt = ps.tile([1, N], F32, tag="ps", name="ps")
            for cic in range(CIC):
                for o, f in NH:
                    mm(pt[:, o:o + f], wbar_[:, cic:cic + 1],
                       yv[:, cic, o:o + f], cic == 0, cic == CIC - 1)
            q_row = rtmp.tile([1, N], F32, tag="r_q", name="q_row", bufs=2)
            nc.scalar.copy(q_row[:], pt[:])
            p4 = rtmp.tile([1, N], F32, tag="rt", name="p4")
            nc.vector.tensor_mul(p4[:], p3[:], q_row[:])
            for j in range(MC):
                tp_ = ps.tile([128, 1], F32, tag="ps", name="tp_")
                nc.tensor.transpose(tp_[:],
                                    p4[:, j * 128:(j + 1) * 128],
                                    ident[:1, :1])
                nc.vector.tensor_add(pooledT[:, j, s:s + 1], tp_[:],
                                     rscol[:, j:j + 1])

        pending = None
        for s in range(BPC):
            x_t, y_t = in_tiles[s]
            streams = (("x", x_t), ("y", y_t))

            # -- per-stream projection blocks: theta, phi, g --
            pj = {}
            gT = {}
            rscol = rtmp.tile([128, MC], F32, tag="rscol", name="rscol", bufs=2)
            for st, src in streams:
                for pr in ("t", "p"):
                    w = w_sb[f"wt_{pr}{st}"]
                    dst = proj.tile([128, CIC, N], BF16, tag=f"pj_{pr}{st}",
                                    name=f"pj_{pr}{st}")
                    pj[pr + st] = dst
                    for cic in range(CIC):
                        pt = ps.tile([128, N], F32, tag="ps", name="ps")
                        for k in range(KC):
                            for o, f in NH:
                                mm(pt[:, o:o + f],
                                   w[:, k, cic * 128:(cic + 1) * 128],
                                   src[:, k, o:o + f], k == 0, k == KC - 1)
                        nc.scalar.activation(
                            dst[:, cic, :], pt[:], AF.Identity,
                            bias=tb_sb[f"b_{pr}{st}"][:, cic:cic + 1])
                # g projection, (N, CI) layout; col CI carries the
                # gamma/C-scaled residual colsum of this stream
                w = w_sb[f"wt_g{st}"]
                dst = proj.tile([128, MC, CI], BF16, tag=f"gT{st}", name=f"gT{st}")
                gT[st] = dst
                for mc_ in range(MC):
                    pt = ps.tile([128, CI + 1], F32, tag="ps", name="ps")
                    has_b = st in gb_sb
                    for k in range(KC):
                        mm(pt[:], src[:, k, mc_ * 128:(mc_ + 1) * 128],
                           w[:, k, :], k == 0, (k == KC - 1) and not has_b)
                    if has_b:
                        mm(pt[:, :CI], ones_row[:], gb_sb[st][:], False, True)
                    nc.vector.tensor_copy(dst[:, mc_, :], pt[:, :CI])
                    if st == "x":
                        nc.scalar.copy(rscol[:, mc_:mc_ + 1], pt[:, CI:CI + 1])
                    else:
                        nc.vector.tensor_add(rscol[:, mc_:mc_ + 1],
                                             rscol[:, mc_:mc_ + 1],
                                             pt[:, CI:CI + 1])
                if st == "x" and pending is not None:
                    # previous sample's deferred qraw + fixup tail: PE has a
                    # deep queue of projection matmuls above, so the DVE/ACT
                    # dependencies are long resolved when PE reaches these.
                    emit_fixup_tail(pending)
                    pending = None

            if s + 1 < BPC:
                in_tiles.append(load_inputs(s + 1))

            # -- transposed logits + exp --
            E = {st: attp.tile([128, MC, N], BF16, tag=f"E{st}", name=f"E{st}")
                 for st, _ in streams}
            S = attp.tile([128, MC, N], BF16, tag="S", name="S")
            for mc_ in range(MC):
                for st, _ in streams:
                    pt = ps.tile([128, N], F32, tag="ps", name="ps")
                    for cic in range(CIC):
                        for o, f in NH:
                            mm(pt[:, o:o + f],
                               pj["p" + st][:, cic, mc_ * 128:(mc_ + 1) * 128],
                               pj["t" + st][:, cic, o:o + f],
                               cic == 0, cic == CIC - 1)
                    nc.scalar.activation(E[st][:, mc_, :], pt[:], AF.Exp,
                                         bias=expb[:])
                nc.vector.tensor_mul(S[:, mc_, :], E["x"][:, mc_, :],
                                     E["y"][:, mc_, :])

            # -- softmax denominators (partition sums via ones-matmul) --
            zrows = {}
            for key, st in (("zx", "x"), ("zy", "y")):
                pt = ps.tile([1, N], F32, tag="ps", name="ps")
                for mc_ in range(MC):
                    for o, f in NH:
                        mm(pt[:, o:o + f], ones_col[:], E[st][:, mc_, o:o + f],
                           mc_ == 0, mc_ == MC - 1)
                r = rtmp.tile([1, N], F32, tag=f"r_{key}", name=f"r_{key}", bufs=2)
                nc.scalar.copy(r[:], pt[:])
                zrows[key] = r

            # chain A of the fixup: R2 = 1/(Zx*Zy)^2, overlapped with U matmuls
            p1 = rtmp.tile([1, N], F32, tag="rt", name="p1")
            nc.vector.tensor_mul(p1[:], zrows["zx"][:], zrows["zy"][:])
            p2 = rtmp.tile([1, N], F32, tag="rt", name="p2")
            nc.vector.reciprocal(p2[:], p1[:])
            p3 = rtmp.tile([1, N], F32, tag="rt", name="p3")
            nc.vector.tensor_mul(p3[:], p2[:], p2[:])

            # -- unnormalized attention-apply + product --
            yv = attp.tile([128, CIC, N], BF16, tag="yv", name="yv")
            for cic in range(CIC):
                ptu = {}
                for st, _ in streams:
                    pt = ps.tile([128, N], F32, tag="ps", name="ps")
                    ptu[st] = pt
                    for mc_ in range(MC):
                        for o, f in NH:
                            mm(pt[:, o:o + f],
                               gT[st][:, mc_, cic * 128:(cic + 1) * 128],
                               S[:, mc_, o:o + f], mc_ == 0, mc_ == MC - 1)
                # DVE tensor_tensor cannot read two PSUM operands; bounce Ux
                ux_sb = rtmp.tile([128, N], BF16, tag="ux_sb", name="ux_sb", bufs=2)
                nc.scalar.copy(ux_sb[:], ptu["x"][:])
                nc.vector.tensor_mul(yv[:, cic, :], ux_sb[:], ptu["y"][:])

            pending = (s, p3, yv, wbar, rscol)

        emit_fixup_tail(pending)

        # ---- head ----
        pt = ps.tile([BPC, HOUT], F32, tag="ps", name="head_ps")
        for j in range(MC):
            mm(pt[:], pooledT[:, j, :], hwT[:, j, :],
               j == 0, (j == MC - 1) and not has_hb)
        if has_hb:
            mm(pt[:], ones_row[:, :BPC], hb[:], False, True)
        out_sb = rows.tile([BPC, HOUT], F32, tag="out_sb", name="out_sb")
        nc.scalar.copy(out_sb[:], pt[:])
        nc.sync.dma_start(d_out[:], out_sb[:])

    nc.compile()
    return nc


def _prepare(inputs):
    f = lambda k: np.ascontiguousarray(np.asarray(inputs[k], dtype=np.float32))
    bf = lambda a: np.ascontiguousarray(np.asarray(a, dtype=ml_dtypes.bfloat16))
    sar, opt = f("sar"), f("opt")
    ga = float(np.asarray(inputs["gamma_att"]).reshape(-1)[0])
    go = float(np.asarray(inputs["gamma_opt"]).reshape(-1)[0])
    gs = float(np.asarray(inputs["gamma_sar"]).reshape(-1)[0])
    W_w, W_b = f("W_w"), f("W_b")
    head_w, head_b = f("head_w"), f("head_b")

    wbar = (ga / C) * W_w.sum(axis=0)  # (CI,)
    bbar = (ga / C) * float(W_b.sum())
    # fold the pooled-constant through the head: out += bbar * head_w.sum(1)
    hb_eff = head_b + bbar * head_w.sum(axis=1)  # (HOUT,)

    gb_x, gb_y = f("g_sar_b"), f("g_opt_b")
    has_gb_x = bool(np.any(gb_x))
    has_gb_y = bool(np.any(gb_y))
    has_hb = bool(np.any(hb_eff))

    key = (has_gb_x, has_gb_y, has_hb)
    if key not in _cached:
        _cached[key] = _build(*key)
    nc = _cached[key]

    # pack inputs: (B, C, N) -> per-core (BPC, 128, KC*N) partition-major
    def pack_in(a):
        a = a.reshape(B, KC, 128, N).transpose(0, 2, 1, 3).reshape(B, 128, KC * N)
        return np.ascontiguousarray(a).astype(ml_dtypes.bfloat16)

    sar_p, opt_p = pack_in(sar), pack_in(opt)

    common = {
        "wt_tx": _pack(f("theta_sar_w").T),
        "wt_px": _pack(f("phi_sar_w").T),
        "wt_ty": _pack(f("theta_opt_w").T),
        "wt_py": _pack(f("phi_opt_w").T),
        "wt_gx": _pack(np.concatenate(
            [f("g_sar_w").T, np.full((C, 1), gs / C, np.float32)], axis=1)),
        "wt_gy": _pack(np.concatenate(
            [f("g_opt_w").T, np.full((C, 1), go / C, np.float32)], axis=1)),
        "hwT": _pack(head_w.T),
        "wbar": bf(wbar),
        "b_tx": f("theta_sar_b"), "b_px": f("phi_sar_b"),
        "b_ty": f("theta_opt_b"), "b_py": f("phi_opt_b"),
        "ones_col": np.ones((128, 1), ml_dtypes.bfloat16),
        "ident": np.eye(4, dtype=np.float32),
        "expb": np.full((128, 1), EXP_SHIFT, np.float32),
    }
    if has_gb_x or has_gb_y or has_hb:
        common["ones_row"] = np.ones((1, 128), ml_dtypes.bfloat16)
    if has_gb_x:
        common["gb_x"] = bf(gb_x.reshape(1, CI))
    if has_gb_y:
        common["gb_y"] = bf(gb_y.reshape(1, CI))
    if has_hb:
        common["hb"] = bf(hb_eff.reshape(1, HOUT))

    in_maps = []
    for c in range(NCORES):
        m = dict(common)
        m["sar"] = np.ascontiguousarray(sar_p[c * BPC:(c + 1) * BPC])
        m["opt"] = np.ascontiguousarray(opt_p[c * BPC:(c + 1) * BPC])
        in_maps.append(m)
    return nc, in_maps


def kernel(**inputs):
    nc, in_maps = _prepare(inputs)
    res = run_bass_kernel_spmd(nc, in_maps, core_ids=list(range(NCORES)))
    return np.concatenate([res.results[c]["out"] for c in range(NCORES)], axis=0)


if __name__ == "__main__":
    rng = np.random.default_rng(0)
    ins = {
        "sar": rng.standard_normal((B, C, N), dtype=np.float32),
        "opt": rng.standard_normal((B, C, N), dtype=np.float32),
    }
    for nm in ("g_sar", "g_opt", "theta_sar", "theta_opt", "phi_sar", "phi_opt"):
        ins[nm + "_w"] = 0.02 * rng.standard_normal((CI, C), dtype=np.float32)
        ins[nm + "_b"] = np.zeros((CI,), np.float32)
    ins["W_w"] = 0.02 * rng.standard_normal((C, CI), dtype=np.float32)
    ins["W_b"] = np.zeros((C,), np.float32)
    ins["head_w"] = 0.02 * rng.standard_normal((HOUT, N), dtype=np.float32)
    ins["head_b"] = np.zeros((HOUT,), np.float32)
    ins["gamma_sar"] = np.asarray([0.3], np.float32)
    ins["gamma_opt"] = np.asarray([1.0], np.float32)
    ins["gamma_att"] = np.asarray([1.0], np.float32)
    out = kernel(**ins)
    print(out.shape, out.dtype, np.abs(out).mean())


# revision 17
# speedup vs baseline: 1.0571x; 1.0571x over previous
"""Trainium2 Bass kernel for nn_CAFF_3100966388292.

Dual-stream (SAR/OPT) cross-attention fusion net:
  theta/phi/g 1x1-conv projections on both streams, per-sample NxN attention
  maps fused elementwise, both value streams attended, product taken, output
  1x1-conv + residual + channel-mean pool + linear head.

Strategy (pure data parallel, 4 samples per core on 8 cores):
  * Layouts chosen so no on-device transposes of big tensors are needed:
      - theta/phi in (CI, N) layout  (lhsT = host-pretransposed weights)
      - g directly in (N, CI) layout (lhsT = input tile, rhs = w^T)
      - attention logits computed TRANSPOSED: L^T(m,n) = phi^T theta, so the
        contracted dim (m) of att@g lands on PSUM partitions naturally.
  * Softmax denominators via ones-column matmuls on the tensor engine
    (partition-dim sums), applied as a scalar fixup on the pooled row:
      (att@g_x * att@g_y)(n,:) = Ux(:,n)*Uy(:,n) / (Zx(n)*Zy(n))^2
    with U the unnormalized attended values (global EXP_SHIFT cancels too).
  * The final W-projection + residual + channel-mean + head collapse
    algebraically:
      pooled(n) = R2(n)*qraw(n) + (ga/C)*sum(W_b) + rs(n),
      rs(n)     = (go/C)*colsum(opt)(n) + (gs/C)*colsum(sar)(n)  [one PSUM acc]
      qraw(n)   = sum_ci wbar(ci) * Ux(ci,n) * Uy(ci,n),
      wbar      = (ga/C) * W_w.sum(0)
    which removes the (C,CI)x(CI,N) W matmul entirely.
  * bf16 on all matmul operands (host-casts + host-packs inputs/weights into
    partition-major layout for single large contiguous-line DMAs). PSUM and
    the pooled fixup chain stay fp32. Final rel err ~3e-3, dominated by bf16
    rounding of the residual colsum path.
  * The per-sample fixup chain + pooled-row transposes are emitted deferred
    (inside the next sample's projection phase) so the PE never stalls on the
    serial DVE row chain.
"""

import sys
import types

import ml_dtypes
import numpy as np

# The agent image's antenv package lacks axon_hooks; register the equivalent
# NTFF hook so run_bass_kernel_spmd(trace=True) works if ever requested.
try:  # pragma: no cover
    import antenv.axon_hooks  # noqa: F401
except ImportError:
    try:
        from trn_agent_boot.trn_boot import _ntff_profile_via_ctypes

        _hook = _ntff_profile_via_ctypes("/opt/axon/libaxon_pjrt.so")
        _mod = types.ModuleType("antenv.axon_hooks")
        _mod.get_axon_ntff_profile_hook = lambda: _hook
        _mod.set_axon_ntff_profile_hook = lambda h: None
        sys.modules["antenv.axon_hooks"] = _mod
    except Exception:
        pass

import concourse.bass as bass
import concourse.tile as tile
from concourse import bacc, mybir
from concourse.bass_utils import run_bass_kernel_spmd

F32 = mybir.dt.float32
BF16 = mybir.dt.bfloat16
EXP_SHIFT = -12.0  # constant logit shift before exp; cancels exactly in the math

B, C, CI, N, HOUT = 32, 512, 256, 768, 256
NCORES = 8
BPC = B // NCORES  # samples per core
KC = C // 128  # 4 k-chunks over channels
MC = N // 128  # 6 chunks over positions
CIC = CI // 128  # 2 chunks over inner channels
# free-dim split of N into PSUM-bank-legal matmul halves
NH = ((0, 512), (512, 256))

_cached = {}


def _pack(a):
    """(R, F) host array -> (128, R//128 * F) partition-major bf16."""
    a = np.asarray(a, dtype=np.float32)
    r, f = a.shape
    k = r // 128
    return np.ascontiguousarray(
        a.reshape(k, 128, f).transpose(1, 0, 2).reshape(128, k * f)
    ).astype(ml_dtypes.bfloat16)


def _build(has_gb_x, has_gb_y, has_hb):
    nc = bacc.Bacc("TRN2", target_bir_lowering=False, debug=False)
    AF = mybir.ActivationFunctionType

    def mm(out, lhsT, rhs, start, stop):
        nc.tensor.matmul(out, lhsT, rhs, start=start, stop=stop)

    # inputs host-packed to (BPC, 128, KC*N) partition-major bf16
    d_sar = nc.dram_tensor("sar", [BPC, 128, KC * N], BF16, kind="ExternalInput")
    d_opt = nc.dram_tensor("opt", [BPC, 128, KC * N], BF16, kind="ExternalInput")
    # host-pretransposed + packed projection weights, (128, KC*CI) bf16
    d_w = {}
    for nm in ("wt_tx", "wt_px", "wt_ty", "wt_py"):
        d_w[nm] = nc.dram_tensor(nm, [128, KC * CI], BF16, kind="ExternalInput")
    for nm in ("wt_gx", "wt_gy"):  # g weights carry a gamma/C ones column
        d_w[nm] = nc.dram_tensor(nm, [128, KC * (CI + 1)], BF16,
                                 kind="ExternalInput")
    d_hwT = nc.dram_tensor("hwT", [128, MC * HOUT], BF16, kind="ExternalInput")
    d_wbar = nc.dram_tensor("wbar", [CI], BF16, kind="ExternalInput")
    d_tb = {  # theta/phi per-partition bias columns (CI,), fp32 (ACT bias)
        nm: nc.dram_tensor(nm, [CI], F32, kind="ExternalInput")
        for nm in ("b_tx", "b_px", "b_ty", "b_py")
    }
    d_ones = nc.dram_tensor("ones_col", [128, 1], BF16, kind="ExternalInput")
    need_onesr = has_gb_x or has_gb_y or has_hb
    if need_onesr:
        d_onesr = nc.dram_tensor("ones_row", [1, 128], BF16, kind="ExternalInput")
    d_ident = nc.dram_tensor("ident", [4, 4], F32, kind="ExternalInput")
    d_expb = nc.dram_tensor("expb", [128, 1], F32, kind="ExternalInput")
    d_gb = {}
    if has_gb_x:
        d_gb["x"] = nc.dram_tensor("gb_x", [1, CI], BF16, kind="ExternalInput")
    if has_gb_y:
        d_gb["y"] = nc.dram_tensor("gb_y", [1, CI], BF16, kind="ExternalInput")
    if has_hb:
        d_hb = nc.dram_tensor("hb", [1, HOUT], BF16, kind="ExternalInput")
    d_out = nc.dram_tensor("out", [BPC, HOUT], F32, kind="ExternalOutput")

    with tile.TileContext(nc) as tc, \
            tc.tile_pool(name="wts", bufs=1) as wts, \
            tc.tile_pool(name="inp", bufs=2) as inp, \
            tc.tile_pool(name="proj", bufs=1) as proj, \
            tc.tile_pool(name="att", bufs=1) as attp, \
            tc.tile_pool(name="rows", bufs=1) as rows, \
            tc.tile_pool(name="rtmp", bufs=4) as rtmp, \
            tc.tile_pool(name="ps", bufs=4, space="PSUM") as ps:

        # ---- DMAs in strict first-use order: the queues are FIFO, so
        # everything emitted ahead of the first matmul's dependencies delays
        # kernel start ----
        def load_w(nm, cols=CI):
            t = wts.tile([128, KC, cols], BF16, tag=nm, name=nm)
            nc.sync.dma_start(t[:], d_w[nm].ap().rearrange("p (k f) -> p k f", k=KC))
            return t

        w_sb = {}
        w_sb["wt_tx"] = load_w("wt_tx")
        x0 = inp.tile([128, KC, N], BF16, tag="x_t", name="x_t")
        for k in range(KC):
            nc.sync.dma_start(x0[:, k, :], d_sar[0][:, k * N:(k + 1) * N])
        w_sb["wt_px"] = load_w("wt_px")
        w_sb["wt_gx"] = load_w("wt_gx", CI + 1)
        tb_sb = {}
        for nm, d in d_tb.items():
            t = wts.tile([128, CIC], F32, tag=nm, name=nm)
            nc.sync.dma_start(t[:], d.ap().rearrange("(k p) -> p k", p=128))
            tb_sb[nm] = t
        w_sb["wt_ty"] = load_w("wt_ty")
        w_sb["wt_py"] = load_w("wt_py")
        w_sb["wt_gy"] = load_w("wt_gy", CI + 1)
        y0 = inp.tile([128, KC, N], BF16, tag="y_t", name="y_t")
        for k in range(KC):
            nc.sync.dma_start(y0[:, k, :], d_opt[0][:, k * N:(k + 1) * N])

        def load_inputs(s):
            x_t = inp.tile([128, KC, N], BF16, tag="x_t", name="x_t")
            y_t = inp.tile([128, KC, N], BF16, tag="y_t", name="y_t")
            for k in range(KC):
                nc.sync.dma_start(x_t[:, k, :], d_sar[s][:, k * N:(k + 1) * N])
            for k in range(KC):
                nc.sync.dma_start(y_t[:, k, :], d_opt[s][:, k * N:(k + 1) * N])
            return x_t, y_t

        in_tiles = [(x0, y0)]

        # ---- small constants (all needed later than the projections) ----
        wbar = wts.tile([128, CIC], BF16, tag="wbar", name="wbar")
        nc.sync.dma_start(wbar[:], d_wbar.ap().rearrange("(k p) -> p k", p=128))
        ones_col = wts.tile([128, 1], BF16, tag="ones_col", name="ones_col")
        nc.sync.dma_start(ones_col[:], d_ones.ap())
        ident = wts.tile([4, 4], F32, tag="ident", name="ident")
        nc.sync.dma_start(ident[:], d_ident.ap())
        expb = wts.tile([128, 1], F32, tag="expb", name="expb")
        nc.sync.dma_start(expb[:], d_expb.ap())
        hwT = wts.tile([128, MC, HOUT], BF16, tag="hwT", name="hwT")
        nc.sync.dma_start(hwT[:], d_hwT.ap().rearrange("p (k f) -> p k f", k=MC))
        if need_onesr:
            ones_row = wts.tile([1, 128], BF16, tag="ones_row", name="ones_row")
            nc.sync.dma_start(ones_row[:], d_onesr.ap())
        gb_sb = {}
        for st, d in d_gb.items():
            t = wts.tile([1, CI], BF16, tag=f"gb_{st}", name=f"gb_{st}")
            nc.sync.dma_start(t[:], d.ap())
            gb_sb[st] = t
        if has_hb:
            hb = wts.tile([1, HOUT], BF16, tag="hb", name="hb")
            nc.sync.dma_start(hb[:], d_hb.ap())

        pooledT = rows.tile([128, MC, BPC], BF16, tag="pooledT", name="pooledT")

        def emit_fixup_qraw(fx):
            """qraw matvec + chain B (PE then ACT/DVE latency off PE path)."""
            s, p3, yv, wbar_, rscol = fx
            pt = ps.tile([1, N], F32, tag="ps", name="ps")
            for cic in range(CIC):
                for o, f in NH:
                    mm(pt[:, o:o + f], wbar_[:, cic:cic + 1],
                       yv[:, cic, o:o + f], cic == 0, cic == CIC - 1)
            q_row = rtmp.tile([1, N], F32, tag="r_q", name="q_row", bufs=2)
            nc.scalar.copy(q_row[:], pt[:])
            p4 = rtmp.tile([1, N], F32, tag="rt", name="p4")
            nc.vector.tensor_mul(p4[:], p3[:], q_row[:])
            return (s, p4, rscol)

        def emit_fixup_transposes(fx):
            s, p4, rscol = fx
            for j in range(MC):
                tp_ = ps.tile([128, 1], F32, tag="ps", name="tp_")
                nc.tensor.transpose(tp_[:],
                                    p4[:, j * 128:(j + 1) * 128],
                                    ident[:1, :1])
                nc.vector.tensor_add(pooledT[:, j, s:s + 1], tp_[:],
                                     rscol[:, j:j + 1])

        pending = None
        pending_t = None
        for s in range(BPC):
            x_t, y_t = in_tiles[s]
            streams = (("x", x_t), ("y", y_t))

            # -- per-stream projection blocks: theta, phi, g --
            pj = {}
            gT = {}
            rscol = rtmp.tile([128, MC], F32, tag="rscol", name="rscol", bufs=2)
            for st, src in streams:
                for pr in ("t", "p"):
                    w = w_sb[f"wt_{pr}{st}"]
                    dst = proj.tile([128, CIC, N], BF16, tag=f"pj_{pr}{st}",
                                    name=f"pj_{pr}{st}")
                    pj[pr + st] = dst
                    for cic in range(CIC):
                        pt = ps.tile([128, N], F32, tag="ps", name="ps")
                        for k in range(KC):
                            for o, f in NH:
                                mm(pt[:, o:o + f],
                                   w[:, k, cic * 128:(cic + 1) * 128],
                                   src[:, k, o:o + f], k == 0, k == KC - 1)
                        nc.scalar.activation(
                            dst[:, cic, :], pt[:], AF.Identity,
                            bias=tb_sb[f"b_{pr}{st}"][:, cic:cic + 1])
                # g projection, (N, CI) layout; col CI carries the
                # gamma/C-scaled residual colsum of this stream
                w = w_sb[f"wt_g{st}"]
                dst = proj.tile([128, MC, CI], BF16, tag=f"gT{st}", name=f"gT{st}")
                gT[st] = dst
                for mc_ in range(MC):
                    pt = ps.tile([128, CI + 1], F32, tag="ps", name="ps")
                    has_b = st in gb_sb
                    for k in range(KC):
                        mm(pt[:], src[:, k, mc_ * 128:(mc_ + 1) * 128],
                           w[:, k, :], k == 0, (k == KC - 1) and not has_b)
                    if has_b:
                        mm(pt[:, :CI], ones_row[:], gb_sb[st][:], False, True)
                    nc.vector.tensor_copy(dst[:, mc_, :], pt[:, :CI])
                    if st == "x":
                        nc.scalar.copy(rscol[:, mc_:mc_ + 1], pt[:, CI:CI + 1])
                    else:
                        nc.vector.tensor_add(rscol[:, mc_:mc_ + 1],
                                             rscol[:, mc_:mc_ + 1],
                                             pt[:, CI:CI + 1])
                # previous sample's deferred fixup, staged so PE never
                # waits on the ACT/DVE row chain: qraw after the x-stream
                # block, transposes a full stream block later.
                if st == "x" and pending is not None:
                    pending_t = emit_fixup_qraw(pending)
                    pending = None
                elif st == "y" and pending_t is not None:
                    emit_fixup_transposes(pending_t)
                    pending_t = None

            if s + 1 < BPC:
                in_tiles.append(load_inputs(s + 1))

            # -- transposed logits + exp --
            E = {st: attp.tile([128, MC, N], BF16, tag=f"E{st}", name=f"E{st}")
                 for st, _ in streams}
            S = attp.tile([128, MC, N], BF16, tag="S", name="S")
            for mc_ in range(MC):
                for st, _ in streams:
                    pt = ps.tile([128, N], F32, tag="ps", name="ps")
                    for cic in range(CIC):
                        for o, f in NH:
                            mm(pt[:, o:o + f],
                               pj["p" + st][:, cic, mc_ * 128:(mc_ + 1) * 128],
                               pj["t" + st][:, cic, o:o + f],
                               cic == 0, cic == CIC - 1)
                    nc.scalar.activation(E[st][:, mc_, :], pt[:], AF.Exp,
                                         bias=expb[:])
                nc.vector.tensor_mul(S[:, mc_, :], E["x"][:, mc_, :],
                                     E["y"][:, mc_, :])

            # -- softmax denominators (partition sums via ones-matmul) --
            zrows = {}
            for key, st in (("zx", "x"), ("zy", "y")):
                pt = ps.tile([1, N], F32, tag="ps", name="ps")
                for mc_ in range(MC):
                    for o, f in NH:
                        mm(pt[:, o:o + f], ones_col[:], E[st][:, mc_, o:o + f],
                           mc_ == 0, mc_ == MC - 1)
                r = rtmp.tile([1, N], F32, tag=f"r_{key}", name=f"r_{key}", bufs=2)
                nc.scalar.copy(r[:], pt[:])
                zrows[key] = r

            # chain A of the fixup: R2 = 1/(Zx*Zy)^2, overlapped with U matmuls
            p1 = rtmp.tile([1, N], F32, tag="rt", name="p1")
            nc.vector.tensor_mul(p1[:], zrows["zx"][:], zrows["zy"][:])
            p2 = rtmp.tile([1, N], F32, tag="rt", name="p2")
            nc.vector.reciprocal(p2[:], p1[:])
            p3 = rtmp.tile([1, N], F32, tag="rt", name="p3")
            nc.vector.tensor_mul(p3[:], p2[:], p2[:])

            # -- unnormalized attention-apply + product --
            yv = attp.tile([128, CIC, N], BF16, tag="yv", name="yv")
            for cic in range(CIC):
                ptu = {}
                for st, _ in streams:
                    pt = ps.tile([128, N], F32, tag="ps", name="ps")
                    ptu[st] = pt
                    for mc_ in range(MC):
                        for o, f in NH:
                            mm(pt[:, o:o + f],
                               gT[st][:, mc_, cic * 128:(cic + 1) * 128],
                               S[:, mc_, o:o + f], mc_ == 0, mc_ == MC - 1)
                # DVE tensor_tensor cannot read two PSUM operands; bounce Ux
                ux_sb = rtmp.tile([128, N], BF16, tag="ux_sb", name="ux_sb", bufs=2)
                nc.scalar.copy(ux_sb[:], ptu["x"][:])
                nc.vector.tensor_mul(yv[:, cic, :], ux_sb[:], ptu["y"][:])

            pending = (s, p3, yv, wbar, rscol)

        emit_fixup_transposes(emit_fixup_qraw(pending))

        # ---- head ----
        pt = ps.tile([BPC, HOUT], F32, tag="ps", name="head_ps")
        for j in range(MC):
            mm(pt[:], pooledT[:, j, :], hwT[:, j, :],
               j == 0, (j == MC - 1) and not has_hb)
        if has_hb:
            mm(pt[:], ones_row[:, :BPC], hb[:], False, True)
        out_sb = rows.tile([BPC, HOUT], F32, tag="out_sb", name="out_sb")
        nc.scalar.copy(out_sb[:], pt[:])
        nc.sync.dma_start(d_out[:], out_sb[:])

    nc.compile()
    return nc


def _prepare(inputs):
    f = lambda k: np.ascontiguousarray(np.asarray(inputs[k], dtype=np.float32))
    bf = lambda a: np.ascontiguousarray(np.asarray(a, dtype=ml_dtypes.bfloat16))
    sar, opt = f("sar"), f("opt")
    ga = float(np.asarray(inputs["gamma_att"]).reshape(-1)[0])
    go = float(np.asarray(inputs["gamma_opt"]).reshape(-1)[0])
    gs = float(np.asarray(inputs["gamma_sar"]).reshape(-1)[0])
    W_w, W_b = f("W_w"), f("W_b")
    head_w, head_b = f("head_w"), f("head_b")

    wbar = (ga / C) * W_w.sum(axis=0)  # (CI,)
    bbar = (ga / C) * float(W_b.sum())
    # fold the pooled-constant through the head: out += bbar * head_w.sum(1)
    hb_eff = head_b + bbar * head_w.sum(axis=1)  # (HOUT,)

    gb_x, gb_y = f("g_sar_b"), f("g_opt_b")
    has_gb_x = bool(np.any(gb_x))
    has_gb_y = bool(np.any(gb_y))
    has_hb = bool(np.any(hb_eff))

    key = (has_gb_x, has_gb_y, has_hb)
    if key not in _cached:
        _cached[key] = _build(*key)
    nc = _cached[key]

    # pack inputs: (B, C, N) -> per-core (BPC, 128, KC*N) partition-major
    def pack_in(a):
        a = a.reshape(B, KC, 128, N).transpose(0, 2, 1, 3).reshape(B, 128, KC * N)
        return np.ascontiguousarray(a).astype(ml_dtypes.bfloat16)

    sar_p, opt_p = pack_in(sar), pack_in(opt)

    common = {
        "wt_tx": _pack(f("theta_sar_w").T),
        "wt_px": _pack(f("phi_sar_w").T),
        "wt_ty": _pack(f("theta_opt_w").T),
        "wt_py": _pack(f("phi_opt_w").T),
        "wt_gx": _pack(np.concatenate(
            [f("g_sar_w").T, np.full((C, 1), gs / C, np.float32)], axis=1)),
        "wt_gy": _pack(np.concatenate(
            [f("g_opt_w").T, np.full((C, 1), go / C, np.float32)], axis=1)),
        "hwT": _pack(head_w.T),
        "wbar": bf(wbar),
        "b_tx": f("theta_sar_b"), "b_px": f("phi_sar_b"),
        "b_ty": f("theta_opt_b"), "b_py": f("phi_opt_b"),
        "ones_col": np.ones((128, 1), ml_dtypes.bfloat16),
        "ident": np.eye(4, dtype=np.float32),
        "expb": np.full((128, 1), EXP_SHIFT, np.float32),
    }
    if has_gb_x or has_gb_y or has_hb:
        common["ones_row"] = np.ones((1, 128), ml_dtypes.bfloat16)
    if has_gb_x:
        common["gb_x"] = bf(gb_x.reshape(1, CI))
    if has_gb_y:
        common["gb_y"] = bf(gb_y.reshape(1, CI))
    if has_hb:
        common["hb"] = bf(hb_eff.reshape(1, HOUT))

    in_maps = []
    for c in range(NCORES):
        m = dict(common)
        m["sar"] = np.ascontiguousarray(sar_p[c * BPC:(c + 1) * BPC])
        m["opt"] = np.ascontiguousarray(opt_p[c * BPC:(c + 1) * BPC])
        in_maps.append(m)
    return nc, in_maps


def kernel(**inputs):
    nc, in_maps = _prepare(inputs)
    res = run_bass_kernel_spmd(nc, in_maps, core_ids=list(range(NCORES)))
    return np.concatenate([res.results[c]["out"] for c in range(NCORES)], axis=0)


if __name__ == "__main__":
    rng = np.random.default_rng(0)
    ins = {
        "sar": rng.standard_normal((B, C, N), dtype=np.float32),
        "opt": rng.standard_normal((B, C, N), dtype=np.float32),
    }
    for nm in ("g_sar", "g_opt", "theta_sar", "theta_opt", "phi_sar", "phi_opt"):
        ins[nm + "_w"] = 0.02 * rng.standard_normal((CI, C), dtype=np.float32)
        ins[nm + "_b"] = np.zeros((CI,), np.float32)
    ins["W_w"] = 0.02 * rng.standard_normal((C, CI), dtype=np.float32)
    ins["W_b"] = np.zeros((C,), np.float32)
    ins["head_w"] = 0.02 * rng.standard_normal((HOUT, N), dtype=np.float32)
    ins["head_b"] = np.zeros((HOUT,), np.float32)
    ins["gamma_sar"] = np.asarray([0.3], np.float32)
    ins["gamma_opt"] = np.asarray([1.0], np.float32)
    ins["gamma_att"] = np.asarray([1.0], np.float32)
    out = kernel(**ins)
    print(out.shape, out.dtype, np.abs(out).mean())


# revision 18
# speedup vs baseline: 1.0592x; 1.0020x over previous
"""Trainium2 Bass kernel for nn_CAFF_3100966388292.

Dual-stream (SAR/OPT) cross-attention fusion net:
  theta/phi/g 1x1-conv projections on both streams, per-sample NxN attention
  maps fused elementwise, both value streams attended, product taken, output
  1x1-conv + residual + channel-mean pool + linear head.

Strategy (pure data parallel, 4 samples per core on 8 cores):
  * Layouts chosen so no on-device transposes of big tensors are needed:
      - theta/phi in (CI, N) layout  (lhsT = host-pretransposed weights)
      - g directly in (N, CI) layout (lhsT = input tile, rhs = w^T)
      - attention logits computed TRANSPOSED: L^T(m,n) = phi^T theta, so the
        contracted dim (m) of att@g lands on PSUM partitions naturally.
  * Softmax denominators via ones-column matmuls on the tensor engine
    (partition-dim sums), applied as a scalar fixup on the pooled row:
      (att@g_x * att@g_y)(n,:) = Ux(:,n)*Uy(:,n) / (Zx(n)*Zy(n))^2
    with U the unnormalized attended values (global EXP_SHIFT cancels too).
  * The final W-projection + residual + channel-mean + head collapse
    algebraically:
      pooled(n) = R2(n)*qraw(n) + (ga/C)*sum(W_b) + rs(n),
      rs(n)     = (go/C)*colsum(opt)(n) + (gs/C)*colsum(sar)(n)  [one PSUM acc]
      qraw(n)   = sum_ci wbar(ci) * Ux(ci,n) * Uy(ci,n),
      wbar      = (ga/C) * W_w.sum(0)
    which removes the (C,CI)x(CI,N) W matmul entirely.
  * bf16 on all matmul operands (host-casts + host-packs inputs/weights into
    partition-major layout for single large contiguous-line DMAs). PSUM and
    the pooled fixup chain stay fp32. Final rel err ~3e-3, dominated by bf16
    rounding of the residual colsum path.
  * The per-sample fixup chain + pooled-row transposes are emitted deferred
    (inside the next sample's projection phase) so the PE never stalls on the
    serial DVE row chain.
"""

import sys
import types

import ml_dtypes
import numpy as np

# The agent image's antenv package lacks axon_hooks; register the equivalent
# NTFF hook so run_bass_kernel_spmd(trace=True) works if ever requested.
try:  # pragma: no cover
    import antenv.axon_hooks  # noqa: F401
except ImportError:
    try:
        from trn_agent_boot.trn_boot import _ntff_profile_via_ctypes

        _hook = _ntff_profile_via_ctypes("/opt/axon/libaxon_pjrt.so")
        _mod = types.ModuleType("antenv.axon_hooks")
        _mod.get_axon_ntff_profile_hook = lambda: _hook
        _mod.set_axon_ntff_profile_hook = lambda h: None
        sys.modules["antenv.axon_hooks"] = _mod
    except Exception:
        pass

import concourse.bass as bass
import concourse.tile as tile
from concourse import bacc, mybir
from concourse.bass_utils import run_bass_kernel_spmd

F32 = mybir.dt.float32
BF16 = mybir.dt.bfloat16
EXP_SHIFT = -12.0  # constant logit shift before exp; cancels exactly in the math

B, C, CI, N, HOUT = 32, 512, 256, 768, 256
NCORES = 8
BPC = B // NCORES  # samples per core
KC = C // 128  # 4 k-chunks over channels
MC = N // 128  # 6 chunks over positions
CIC = CI // 128  # 2 chunks over inner channels
# free-dim split of N into PSUM-bank-legal matmul halves
NH = ((0, 512), (512, 256))

_cached = {}


def _pack(a):
    """(R, F) host array -> (128, R//128 * F) partition-major bf16."""
    a = np.asarray(a, dtype=np.float32)
    r, f = a.shape
    k = r // 128
    return np.ascontiguousarray(
        a.reshape(k, 128, f).transpose(1, 0, 2).reshape(128, k * f)
    ).astype(ml_dtypes.bfloat16)


def _build(has_gb_x, has_gb_y, has_hb):
    nc = bacc.Bacc("TRN2", target_bir_lowering=False, debug=False)
    AF = mybir.ActivationFunctionType

    def mm(out, lhsT, rhs, start, stop):
        nc.tensor.matmul(out, lhsT, rhs, start=start, stop=stop)

    # inputs host-packed to (BPC, 128, KC*N) partition-major bf16
    d_sar = nc.dram_tensor("sar", [BPC, 128, KC * N], BF16, kind="ExternalInput")
    d_opt = nc.dram_tensor("opt", [BPC, 128, KC * N], BF16, kind="ExternalInput")
    # host-pretransposed + packed projection weights, (128, KC*CI) bf16
    d_w = {}
    for nm in ("wt_tx", "wt_px", "wt_ty", "wt_py"):
        d_w[nm] = nc.dram_tensor(nm, [128, KC * CI], BF16, kind="ExternalInput")
    for nm in ("wt_gx", "wt_gy"):  # g weights carry a gamma/C ones column
        d_w[nm] = nc.dram_tensor(nm, [128, KC * (CI + 1)], BF16,
                                 kind="ExternalInput")
    d_hwT = nc.dram_tensor("hwT", [128, MC * HOUT], BF16, kind="ExternalInput")
    d_wbar = nc.dram_tensor("wbar", [CI], BF16, kind="ExternalInput")
    d_tb = {  # theta/phi per-partition bias columns (CI,), fp32 (ACT bias)
        nm: nc.dram_tensor(nm, [CI], F32, kind="ExternalInput")
        for nm in ("b_tx", "b_px", "b_ty", "b_py")
    }
    d_ones = nc.dram_tensor("ones_col", [128, 1], BF16, kind="ExternalInput")
    need_onesr = has_gb_x or has_gb_y or has_hb
    if need_onesr:
        d_onesr = nc.dram_tensor("ones_row", [1, 128], BF16, kind="ExternalInput")
    d_ident = nc.dram_tensor("ident", [4, 4], F32, kind="ExternalInput")
    d_expb = nc.dram_tensor("expb", [128, 1], F32, kind="ExternalInput")
    d_gb = {}
    if has_gb_x:
        d_gb["x"] = nc.dram_tensor("gb_x", [1, CI], BF16, kind="ExternalInput")
    if has_gb_y:
        d_gb["y"] = nc.dram_tensor("gb_y", [1, CI], BF16, kind="ExternalInput")
    if has_hb:
        d_hb = nc.dram_tensor("hb", [1, HOUT], BF16, kind="ExternalInput")
    d_out = nc.dram_tensor("out", [BPC, HOUT], F32, kind="ExternalOutput")

    with tile.TileContext(nc) as tc, \
            tc.tile_pool(name="wts", bufs=1) as wts, \
            tc.tile_pool(name="inp", bufs=2) as inp, \
            tc.tile_pool(name="proj", bufs=1) as proj, \
            tc.tile_pool(name="att", bufs=1) as attp, \
            tc.tile_pool(name="rows", bufs=1) as rows, \
            tc.tile_pool(name="rtmp", bufs=4) as rtmp, \
            tc.tile_pool(name="ps", bufs=4, space="PSUM") as ps:

        # ---- DMAs in strict first-use order: the queues are FIFO, so
        # everything emitted ahead of the first matmul's dependencies delays
        # kernel start ----
        def load_w(nm, cols=CI):
            t = wts.tile([128, KC, cols], BF16, tag=nm, name=nm)
            nc.sync.dma_start(t[:], d_w[nm].ap().rearrange("p (k f) -> p k f", k=KC))
            return t

        w_sb = {}
        # first weight + first input chunk gate the whole kernel: issue the
        # k0 pieces first so matmul #1 waits on ~256KB, not ~1MB
        t = wts.tile([128, KC, CI], BF16, tag="wt_tx", name="wt_tx")
        w_sb["wt_tx"] = t
        nc.sync.dma_start(t[:, 0, :], d_w["wt_tx"].ap()[:, :CI])
        x0 = inp.tile([128, KC, N], BF16, tag="x_t", name="x_t")
        nc.sync.dma_start(x0[:, 0, :], d_sar[0][:, :N])
        nc.sync.dma_start(
            t[:, 1:, :],
            d_w["wt_tx"].ap()[:, CI:].rearrange("p (k f) -> p k f", k=KC - 1))
        for k in range(1, KC):
            nc.sync.dma_start(x0[:, k, :], d_sar[0][:, k * N:(k + 1) * N])
        w_sb["wt_px"] = load_w("wt_px")
        w_sb["wt_gx"] = load_w("wt_gx", CI + 1)
        tb_sb = {}
        for nm, d in d_tb.items():
            t = wts.tile([128, CIC], F32, tag=nm, name=nm)
            nc.sync.dma_start(t[:], d.ap().rearrange("(k p) -> p k", p=128))
            tb_sb[nm] = t
        w_sb["wt_ty"] = load_w("wt_ty")
        w_sb["wt_py"] = load_w("wt_py")
        w_sb["wt_gy"] = load_w("wt_gy", CI + 1)
        y0 = inp.tile([128, KC, N], BF16, tag="y_t", name="y_t")
        for k in range(KC):
            nc.sync.dma_start(y0[:, k, :], d_opt[0][:, k * N:(k + 1) * N])

        def load_inputs(s):
            x_t = inp.tile([128, KC, N], BF16, tag="x_t", name="x_t")
            y_t = inp.tile([128, KC, N], BF16, tag="y_t", name="y_t")
            for k in range(KC):
                nc.sync.dma_start(x_t[:, k, :], d_sar[s][:, k * N:(k + 1) * N])
            for k in range(KC):
                nc.sync.dma_start(y_t[:, k, :], d_opt[s][:, k * N:(k + 1) * N])
            return x_t, y_t

        in_tiles = [(x0, y0)]

        # ---- small constants (all needed later than the projections) ----
        wbar = wts.tile([128, CIC], BF16, tag="wbar", name="wbar")
        nc.sync.dma_start(wbar[:], d_wbar.ap().rearrange("(k p) -> p k", p=128))
        ones_col = wts.tile([128, 1], BF16, tag="ones_col", name="ones_col")
        nc.sync.dma_start(ones_col[:], d_ones.ap())
        ident = wts.tile([4, 4], F32, tag="ident", name="ident")
        nc.sync.dma_start(ident[:], d_ident.ap())
        expb = wts.tile([128, 1], F32, tag="expb", name="expb")
        nc.sync.dma_start(expb[:], d_expb.ap())
        hwT = wts.tile([128, MC, HOUT], BF16, tag="hwT", name="hwT")
        nc.sync.dma_start(hwT[:], d_hwT.ap().rearrange("p (k f) -> p k f", k=MC))
        if need_onesr:
            ones_row = wts.tile([1, 128], BF16, tag="ones_row", name="ones_row")
            nc.sync.dma_start(ones_row[:], d_onesr.ap())
        gb_sb = {}
        for st, d in d_gb.items():
            t = wts.tile([1, CI], BF16, tag=f"gb_{st}", name=f"gb_{st}")
            nc.sync.dma_start(t[:], d.ap())
            gb_sb[st] = t
        if has_hb:
            hb = wts.tile([1, HOUT], BF16, tag="hb", name="hb")
            nc.sync.dma_start(hb[:], d_hb.ap())

        pooledT = rows.tile([128, MC, BPC], BF16, tag="pooledT", name="pooledT")

        def emit_fixup_qraw(fx):
            """qraw matvec + chain B (PE then ACT/DVE latency off PE path)."""
            s, p3, yv, wbar_, rscol = fx
            pt = ps.tile([1, N], F32, tag="ps", name="ps")
            for cic in range(CIC):
                for o, f in NH:
                    mm(pt[:, o:o + f], wbar_[:, cic:cic + 1],
                       yv[:, cic, o:o + f], cic == 0, cic == CIC - 1)
            q_row = rtmp.tile([1, N], F32, tag="r_q", name="q_row", bufs=2)
            nc.scalar.copy(q_row[:], pt[:])
            p4 = rtmp.tile([1, N], F32, tag="rt", name="p4")
            nc.vector.tensor_mul(p4[:], p3[:], q_row[:])
            return (s, p4, rscol)

        def emit_fixup_transposes(fx):
            s, p4, rscol = fx
            for j in range(MC):
                tp_ = ps.tile([128, 1], F32, tag="ps", name="tp_")
                nc.tensor.transpose(tp_[:],
                                    p4[:, j * 128:(j + 1) * 128],
                                    ident[:1, :1])
                nc.vector.tensor_add(pooledT[:, j, s:s + 1], tp_[:],
                                     rscol[:, j:j + 1])

        pending = None
        pending_t = None
        for s in range(BPC):
            x_t, y_t = in_tiles[s]
            streams = (("x", x_t), ("y", y_t))

            # -- per-stream projection blocks: theta, phi, g --
            pj = {}
            gT = {}
            rscol = rtmp.tile([128, MC], F32, tag="rscol", name="rscol", bufs=2)
            for st, src in streams:
                for pr in ("t", "p"):
                    w = w_sb[f"wt_{pr}{st}"]
                    dst = proj.tile([128, CIC, N], BF16, tag=f"pj_{pr}{st}",
                                    name=f"pj_{pr}{st}")
                    pj[pr + st] = dst
                    for cic in range(CIC):
                        pt = ps.tile([128, N], F32, tag="ps", name="ps")
                        for k in range(KC):
                            for o, f in NH:
                                mm(pt[:, o:o + f],
                                   w[:, k, cic * 128:(cic + 1) * 128],
                                   src[:, k, o:o + f], k == 0, k == KC - 1)
                        nc.scalar.activation(
                            dst[:, cic, :], pt[:], AF.Identity,
                            bias=tb_sb[f"b_{pr}{st}"][:, cic:cic + 1])
                # g projection, (N, CI) layout; col CI carries the
                # gamma/C-scaled residual colsum of this stream
                w = w_sb[f"wt_g{st}"]
                dst = proj.tile([128, MC, CI], BF16, tag=f"gT{st}", name=f"gT{st}")
                gT[st] = dst
                for mc_ in range(MC):
                    pt = ps.tile([128, CI + 1], F32, tag="ps", name="ps")
                    has_b = st in gb_sb
                    for k in range(KC):
                        mm(pt[:], src[:, k, mc_ * 128:(mc_ + 1) * 128],
                           w[:, k, :], k == 0, (k == KC - 1) and not has_b)
                    if has_b:
                        mm(pt[:, :CI], ones_row[:], gb_sb[st][:], False, True)
                    nc.vector.tensor_copy(dst[:, mc_, :], pt[:, :CI])
                    if st == "x":
                        nc.scalar.copy(rscol[:, mc_:mc_ + 1], pt[:, CI:CI + 1])
                    else:
                        nc.vector.tensor_add(rscol[:, mc_:mc_ + 1],
                                             rscol[:, mc_:mc_ + 1],
                                             pt[:, CI:CI + 1])
                # previous sample's deferred fixup, staged so PE never
                # waits on the ACT/DVE row chain: qraw after the x-stream
                # block, transposes a full stream block later.
                if st == "x" and pending is not None:
                    pending_t = emit_fixup_qraw(pending)
                    pending = None
                elif st == "y" and pending_t is not None:
                    emit_fixup_transposes(pending_t)
                    pending_t = None

            if s + 1 < BPC:
                in_tiles.append(load_inputs(s + 1))

            # -- transposed logits + exp --
            E = {st: attp.tile([128, MC, N], BF16, tag=f"E{st}", name=f"E{st}")
                 for st, _ in streams}
            S = attp.tile([128, MC, N], BF16, tag="S", name="S")
            for mc_ in range(MC):
                for st, _ in streams:
                    pt = ps.tile([128, N], F32, tag="ps", name="ps")
                    for cic in range(CIC):
                        for o, f in NH:
                            mm(pt[:, o:o + f],
                               pj["p" + st][:, cic, mc_ * 128:(mc_ + 1) * 128],
                               pj["t" + st][:, cic, o:o + f],
                               cic == 0, cic == CIC - 1)
                    nc.scalar.activation(E[st][:, mc_, :], pt[:], AF.Exp,
                                         bias=expb[:])
                nc.vector.tensor_mul(S[:, mc_, :], E["x"][:, mc_, :],
                                     E["y"][:, mc_, :])

            # -- softmax denominators (partition sums via ones-matmul) --
            zrows = {}
            for key, st in (("zx", "x"), ("zy", "y")):
                pt = ps.tile([1, N], F32, tag="ps", name="ps")
                for mc_ in range(MC):
                    for o, f in NH:
                        mm(pt[:, o:o + f], ones_col[:], E[st][:, mc_, o:o + f],
                           mc_ == 0, mc_ == MC - 1)
                r = rtmp.tile([1, N], F32, tag=f"r_{key}", name=f"r_{key}", bufs=2)
                nc.scalar.copy(r[:], pt[:])
                zrows[key] = r

            # chain A of the fixup: R2 = 1/(Zx*Zy)^2, overlapped with U matmuls
            p1 = rtmp.tile([1, N], F32, tag="rt", name="p1")
            nc.vector.tensor_mul(p1[:], zrows["zx"][:], zrows["zy"][:])
            p2 = rtmp.tile([1, N], F32, tag="rt", name="p2")
            nc.vector.reciprocal(p2[:], p1[:])
            p3 = rtmp.tile([1, N], F32, tag="rt", name="p3")
            nc.vector.tensor_mul(p3[:], p2[:], p2[:])

            # -- unnormalized attention-apply + product --
            yv = attp.tile([128, CIC, N], BF16, tag="yv", name="yv")
            for cic in range(CIC):
                ptu = {}
                for st, _ in streams:
                    pt = ps.tile([128, N], F32, tag="ps", name="ps")
                    ptu[st] = pt
                    for mc_ in range(MC):
                        for o, f in NH:
                            mm(pt[:, o:o + f],
                               gT[st][:, mc_, cic * 128:(cic + 1) * 128],
                               S[:, mc_, o:o + f], mc_ == 0, mc_ == MC - 1)
                # DVE tensor_tensor cannot read two PSUM operands; bounce Ux
                ux_sb = rtmp.tile([128, N], BF16, tag="ux_sb", name="ux_sb", bufs=2)
                nc.scalar.copy(ux_sb[:], ptu["x"][:])
                nc.vector.tensor_mul(yv[:, cic, :], ux_sb[:], ptu["y"][:])

            pending = (s, p3, yv, wbar, rscol)

        emit_fixup_transposes(emit_fixup_qraw(pending))

        # ---- head ----
        pt = ps.tile([BPC, HOUT], F32, tag="ps", name="head_ps")
        for j in range(MC):
            mm(pt[:], pooledT[:, j, :], hwT[:, j, :],
               j == 0, (j == MC - 1) and not has_hb)
        if has_hb:
            mm(pt[:], ones_row[:, :BPC], hb[:], False, True)
        out_sb = rows.tile([BPC, HOUT], F32, tag="out_sb", name="out_sb")
        nc.scalar.copy(out_sb[:], pt[:])
        nc.sync.dma_start(d_out[:], out_sb[:])

    nc.compile()
    return nc


def _prepare(inputs):
    f = lambda k: np.ascontiguousarray(np.asarray(inputs[k], dtype=np.float32))
    bf = lambda a: np.ascontiguousarray(np.asarray(a, dtype=ml_dtypes.bfloat16))
    sar, opt = f("sar"), f("opt")
    ga = float(np.asarray(inputs["gamma_att"]).reshape(-1)[0])
    go = float(np.asarray(inputs["gamma_opt"]).reshape(-1)[0])
    gs = float(np.asarray(inputs["gamma_sar"]).reshape(-1)[0])
    W_w, W_b = f("W_w"), f("W_b")
    head_w, head_b = f("head_w"), f("head_b")

    wbar = (ga / C) * W_w.sum(axis=0)  # (CI,)
    bbar = (ga / C) * float(W_b.sum())
    # fold the pooled-constant through the head: out += bbar * head_w.sum(1)
    hb_eff = head_b + bbar * head_w.sum(axis=1)  # (HOUT,)

    gb_x, gb_y = f("g_sar_b"), f("g_opt_b")
    has_gb_x = bool(np.any(gb_x))
    has_gb_y = bool(np.any(gb_y))
    has_hb = bool(np.any(hb_eff))

    key = (has_gb_x, has_gb_y, has_hb)
    if key not in _cached:
        _cached[key] = _build(*key)
    nc = _cached[key]

    # pack inputs: (B, C, N) -> per-core (BPC, 128, KC*N) partition-major
    def pack_in(a):
        a = a.reshape(B, KC, 128, N).transpose(0, 2, 1, 3).reshape(B, 128, KC * N)
        return np.ascontiguousarray(a).astype(ml_dtypes.bfloat16)

    sar_p, opt_p = pack_in(sar), pack_in(opt)

    common = {
        "wt_tx": _pack(f("theta_sar_w").T),
        "wt_px": _pack(f("phi_sar_w").T),
        "wt_ty": _pack(f("theta_opt_w").T),
        "wt_py": _pack(f("phi_opt_w").T),
        "wt_gx": _pack(np.concatenate(
            [f("g_sar_w").T, np.full((C, 1), gs / C, np.float32)], axis=1)),
        "wt_gy": _pack(np.concatenate(
            [f("g_opt_w").T, np.full((C, 1), go / C, np.float32)], axis=1)),
        "hwT": _pack(head_w.T),
        "wbar": bf(wbar),
        "b_tx": f("theta_sar_b"), "b_px": f("phi_sar_b"),
        "b_ty": f("theta_opt_b"), "b_py": f("phi_opt_b"),
        "ones_col": np.ones((128, 1), ml_dtypes.bfloat16),
        "ident": np.eye(4, dtype=np.float32),
        "expb": np.full((128, 1), EXP_SHIFT, np.float32),
    }
    if has_gb_x or has_gb_y or has_hb:
        common["ones_row"] = np.ones((1, 128), ml_dtypes.bfloat16)
    if has_gb_x:
        common["gb_x"] = bf(gb_x.reshape(1, CI))
    if has_gb_y:
        common["gb_y"] = bf(gb_y.reshape(1, CI))
    if has_hb:
        common["hb"] = bf(hb_eff.reshape(1, HOUT))

    in_maps = []
    for c in range(NCORES):
        m = dict(common)
        m["sar"] = np.ascontiguousarray(sar_p[c * BPC:(c + 1) * BPC])
        m["opt"] = np.ascontiguousarray(opt_p[c * BPC:(c + 1) * BPC])
        in_maps.append(m)
    return nc, in_maps


def kernel(**inputs):
    nc, in_maps = _prepare(inputs)
    res = run_bass_kernel_spmd(nc, in_maps, core_ids=list(range(NCORES)))
    return np.concatenate([res.results[c]["out"] for c in range(NCORES)], axis=0)


if __name__ == "__main__":
    rng = np.random.default_rng(0)
    ins = {
        "sar": rng.standard_normal((B, C, N), dtype=np.float32),
        "opt": rng.standard_normal((B, C, N), dtype=np.float32),
    }
    for nm in ("g_sar", "g_opt", "theta_sar", "theta_opt", "phi_sar", "phi_opt"):
        ins[nm + "_w"] = 0.02 * rng.standard_normal((CI, C), dtype=np.float32)
        ins[nm + "_b"] = np.zeros((CI,), np.float32)
    ins["W_w"] = 0.02 * rng.standard_normal((C, CI), dtype=np.float32)
    ins["W_b"] = np.zeros((C,), np.float32)
    ins["head_w"] = 0.02 * rng.standard_normal((HOUT, N), dtype=np.float32)
    ins["head_b"] = np.zeros((HOUT,), np.float32)
    ins["gamma_sar"] = np.asarray([0.3], np.float32)
    ins["gamma_opt"] = np.asarray([1.0], np.float32)
    ins["gamma_att"] = np.asarray([1.0], np.float32)
    out = kernel(**ins)
    print(out.shape, out.dtype, np.abs(out).mean())


# revision 22
# speedup vs baseline: 1.0674x; 1.0077x over previous
"""Trainium2 Bass kernel for nn_CAFF_3100966388292.

Dual-stream (SAR/OPT) cross-attention fusion net:
  theta/phi/g 1x1-conv projections on both streams, per-sample NxN attention
  maps fused elementwise, both value streams attended, product taken, output
  1x1-conv + residual + channel-mean pool + linear head.

Strategy (pure data parallel, 4 samples per core on 8 cores):
  * Layouts chosen so no on-device transposes of big tensors are needed:
      - theta/phi in (CI, N) layout  (lhsT = host-pretransposed weights)
      - g directly in (N, CI) layout (lhsT = input tile, rhs = w^T)
      - attention logits computed TRANSPOSED: L^T(m,n) = phi^T theta, so the
        contracted dim (m) of att@g lands on PSUM partitions naturally.
  * Softmax denominators via ones-column matmuls on the tensor engine
    (partition-dim sums), applied as a scalar fixup on the pooled row:
      (att@g_x * att@g_y)(n,:) = Ux(:,n)*Uy(:,n) / (Zx(n)*Zy(n))^2
    with U the unnormalized attended values (global EXP_SHIFT cancels too).
  * The final W-projection + residual + channel-mean + head collapse
    algebraically:
      pooled(n) = R2(n)*qraw(n) + (ga/C)*sum(W_b) + rs(n),
      rs(n)     = (go/C)*colsum(opt)(n) + (gs/C)*colsum(sar)(n)  [one PSUM acc]
      qraw(n)   = sum_ci wbar(ci) * Ux(ci,n) * Uy(ci,n),
      wbar      = (ga/C) * W_w.sum(0)
    which removes the (C,CI)x(CI,N) W matmul entirely.
  * bf16 on all matmul operands (host-casts + host-packs inputs/weights into
    partition-major layout for single large contiguous-line DMAs). PSUM and
    the pooled fixup chain stay fp32. Final rel err ~3e-3, dominated by bf16
    rounding of the residual colsum path.
  * The per-sample fixup chain + pooled-row transposes are emitted deferred
    (inside the next sample's projection phase) so the PE never stalls on the
    serial DVE row chain.
"""

import sys
import types

import ml_dtypes
import numpy as np

# The agent image's antenv package lacks axon_hooks; register the equivalent
# NTFF hook so run_bass_kernel_spmd(trace=True) works if ever requested.
try:  # pragma: no cover
    import antenv.axon_hooks  # noqa: F401
except ImportError:
    try:
        from trn_agent_boot.trn_boot import _ntff_profile_via_ctypes

        _hook = _ntff_profile_via_ctypes("/opt/axon/libaxon_pjrt.so")
        _mod = types.ModuleType("antenv.axon_hooks")
        _mod.get_axon_ntff_profile_hook = lambda: _hook
        _mod.set_axon_ntff_profile_hook = lambda h: None
        sys.modules["antenv.axon_hooks"] = _mod
    except Exception:
        pass

import concourse.bass as bass
import concourse.tile as tile
from concourse import bacc, mybir
from concourse.bass_utils import run_bass_kernel_spmd

F32 = mybir.dt.float32
BF16 = mybir.dt.bfloat16
FP8 = mybir.dt.float8e4
EXP_SHIFT = -12.0  # constant logit shift before exp; cancels exactly in the math

B, C, CI, N, HOUT = 32, 512, 256, 768, 256
NCORES = 8
BPC = B // NCORES  # samples per core
KC = C // 128  # 4 k-chunks over channels
MC = N // 128  # 6 chunks over positions
CIC = CI // 128  # 2 chunks over inner channels
# free-dim split of N into PSUM-bank-legal matmul halves
NH = ((0, 512), (512, 256))

_cached = {}


def _pack(a):
    """(R, F) host array -> (128, R//128 * F) partition-major bf16."""
    a = np.asarray(a, dtype=np.float32)
    r, f = a.shape
    k = r // 128
    return np.ascontiguousarray(
        a.reshape(k, 128, f).transpose(1, 0, 2).reshape(128, k * f)
    ).astype(ml_dtypes.bfloat16)


def _build(has_gb_x, has_gb_y, has_hb):
    nc = bacc.Bacc("TRN2", target_bir_lowering=False, debug=False)
    AF = mybir.ActivationFunctionType

    def mm(out, lhsT, rhs, start, stop):
        nc.tensor.matmul(out, lhsT, rhs, start=start, stop=stop)

    def mmdr(out, lhsT, rhs, start, stop):
        nc.tensor.matmul(out, lhsT, rhs, start=start, stop=stop,
                         perf_mode=mybir.MatmulPerfMode.DoubleRow)

    # inputs host-packed to (BPC, 128, KC*N) partition-major bf16
    d_sar = nc.dram_tensor("sar", [BPC, 128, KC * N], BF16, kind="ExternalInput")
    d_opt = nc.dram_tensor("opt", [BPC, 128, KC * N], BF16, kind="ExternalInput")
    # host-pretransposed + packed projection weights, (128, KC*CI) bf16
    d_w = {}
    for nm in ("wt_tx", "wt_px", "wt_ty", "wt_py"):
        d_w[nm] = nc.dram_tensor(nm, [128, KC * CI], BF16, kind="ExternalInput")
    for nm in ("wt_gx", "wt_gy"):  # g weights carry a gamma/C ones column
        d_w[nm] = nc.dram_tensor(nm, [128, KC * (CI + 1)], BF16,
                                 kind="ExternalInput")
    d_hwT = nc.dram_tensor("hwT", [128, MC * HOUT], BF16, kind="ExternalInput")
    d_wbar = nc.dram_tensor("wbar", [CI], BF16, kind="ExternalInput")
    d_tb = {  # theta/phi per-partition bias columns (CI,), fp32 (ACT bias)
        nm: nc.dram_tensor(nm, [CI], F32, kind="ExternalInput")
        for nm in ("b_tx", "b_px", "b_ty", "b_py")
    }
    d_ones = nc.dram_tensor("ones_col", [128, 1], BF16, kind="ExternalInput")
    need_onesr = has_gb_x or has_gb_y or has_hb
    if need_onesr:
        d_onesr = nc.dram_tensor("ones_row", [1, 128], BF16, kind="ExternalInput")
    d_ident = nc.dram_tensor("ident", [4, 4], F32, kind="ExternalInput")
    d_expb = nc.dram_tensor("expb", [128, 1], F32, kind="ExternalInput")
    d_gb = {}
    if has_gb_x:
        d_gb["x"] = nc.dram_tensor("gb_x", [1, CI], BF16, kind="ExternalInput")
    if has_gb_y:
        d_gb["y"] = nc.dram_tensor("gb_y", [1, CI], BF16, kind="ExternalInput")
    if has_hb:
        d_hb = nc.dram_tensor("hb", [1, HOUT], BF16, kind="ExternalInput")
    d_out = nc.dram_tensor("out", [BPC, HOUT], F32, kind="ExternalOutput")

    with tile.TileContext(nc) as tc, \
            tc.tile_pool(name="wts", bufs=1) as wts, \
            tc.tile_pool(name="inp", bufs=2) as inp, \
            tc.tile_pool(name="proj", bufs=1) as proj, \
            tc.tile_pool(name="att", bufs=1) as attp, \
            tc.tile_pool(name="rows", bufs=1) as rows, \
            tc.tile_pool(name="rtmp", bufs=4) as rtmp, \
            tc.tile_pool(name="ps", bufs=4, space="PSUM") as ps:

        # ---- DMAs in strict first-use order: the queues are FIFO, so
        # everything emitted ahead of the first matmul's dependencies delays
        # kernel start ----
        def load_w(nm, cols=CI):
            t = wts.tile([128, KC, cols], BF16, tag=nm, name=nm)
            nc.sync.dma_start(t[:], d_w[nm].ap().rearrange("p (k f) -> p k f", k=KC))
            return t

        w_sb = {}
        # first weight + first input chunk gate the whole kernel: issue the
        # k0 pieces first so matmul #1 waits on ~256KB, not ~1MB
        t = wts.tile([128, KC, CI], BF16, tag="wt_tx", name="wt_tx")
        w_sb["wt_tx"] = t
        nc.sync.dma_start(t[:, 0, :], d_w["wt_tx"].ap()[:, :CI])
        x0 = inp.tile([128, KC, N], BF16, tag="x_t", name="x_t")
        nc.sync.dma_start(x0[:, 0, :], d_sar[0][:, :N])
        nc.sync.dma_start(
            t[:, 1:, :],
            d_w["wt_tx"].ap()[:, CI:].rearrange("p (k f) -> p k f", k=KC - 1))
        for k in range(1, KC):
            nc.sync.dma_start(x0[:, k, :], d_sar[0][:, k * N:(k + 1) * N])
        w_sb["wt_px"] = load_w("wt_px")
        w_sb["wt_gx"] = load_w("wt_gx", CI + 1)
        tb_sb = {}
        for nm, d in d_tb.items():
            t = wts.tile([128, CIC], F32, tag=nm, name=nm)
            nc.sync.dma_start(t[:], d.ap().rearrange("(k p) -> p k", p=128))
            tb_sb[nm] = t
        w_sb["wt_ty"] = load_w("wt_ty")
        w_sb["wt_py"] = load_w("wt_py")
        w_sb["wt_gy"] = load_w("wt_gy", CI + 1)
        y0 = inp.tile([128, KC, N], BF16, tag="y_t", name="y_t")
        for k in range(KC):
            nc.sync.dma_start(y0[:, k, :], d_opt[0][:, k * N:(k + 1) * N])

        def load_inputs(s):
            x_t = inp.tile([128, KC, N], BF16, tag="x_t", name="x_t")
            y_t = inp.tile([128, KC, N], BF16, tag="y_t", name="y_t")
            for k in range(KC):
                nc.sync.dma_start(x_t[:, k, :], d_sar[s][:, k * N:(k + 1) * N])
            for k in range(KC):
                nc.sync.dma_start(y_t[:, k, :], d_opt[s][:, k * N:(k + 1) * N])
            return x_t, y_t

        in_tiles = [(x0, y0)]

        # ---- small constants (all needed later than the projections) ----
        wbar = wts.tile([128, CIC], BF16, tag="wbar", name="wbar")
        nc.sync.dma_start(wbar[:], d_wbar.ap().rearrange("(k p) -> p k", p=128))
        ones_col = wts.tile([128, 1], BF16, tag="ones_col", name="ones_col")
        nc.sync.dma_start(ones_col[:], d_ones.ap())
        ident = wts.tile([4, 4], F32, tag="ident", name="ident")
        nc.sync.dma_start(ident[:], d_ident.ap())
        expb = wts.tile([128, 1], F32, tag="expb", name="expb")
        nc.sync.dma_start(expb[:], d_expb.ap())
        hwT = wts.tile([128, MC, HOUT], BF16, tag="hwT", name="hwT")
        nc.sync.dma_start(hwT[:], d_hwT.ap().rearrange("p (k f) -> p k f", k=MC))
        if need_onesr:
            ones_row = wts.tile([1, 128], BF16, tag="ones_row", name="ones_row")
            nc.sync.dma_start(ones_row[:], d_onesr.ap())
        gb_sb = {}
        for st, d in d_gb.items():
            t = wts.tile([1, CI], BF16, tag=f"gb_{st}", name=f"gb_{st}")
            nc.sync.dma_start(t[:], d.ap())
            gb_sb[st] = t
        if has_hb:
            hb = wts.tile([1, HOUT], BF16, tag="hb", name="hb")
            nc.sync.dma_start(hb[:], d_hb.ap())

        pooledT = rows.tile([128, MC, BPC], BF16, tag="pooledT", name="pooledT")

        def emit_fixup_qraw(fx):
            """qraw matvec + chain B (PE then ACT/DVE latency off PE path)."""
            s, p3, yv, wbar_, rscol = fx
            pt = ps.tile([1, N], F32, tag="ps", name="ps")
            for cic in range(CIC):
                for o, f in NH:
                    mm(pt[:, o:o + f], wbar_[:, cic:cic + 1],
                       yv[:, cic, o:o + f], cic == 0, cic == CIC - 1)
            q_row = rtmp.tile([1, N], F32, tag="r_q", name="q_row", bufs=2)
            nc.scalar.copy(q_row[:], pt[:])
            p4 = rtmp.tile([1, N], F32, tag="rt", name="p4")
            nc.vector.tensor_mul(p4[:], p3[:], q_row[:])
            return (s, p4, rscol)

        def emit_fixup_transposes(fx):
            s, p4, rscol = fx
            for j in range(MC):
                tp_ = ps.tile([128, 1], F32, tag="ps", name="tp_")
                nc.tensor.transpose(tp_[:],
                                    p4[:, j * 128:(j + 1) * 128],
                                    ident[:1, :1])
                nc.vector.tensor_add(pooledT[:, j, s:s + 1], tp_[:],
                                     rscol[:, j:j + 1])

        pending = None
        pending_t = None
        for s in range(BPC):
            x_t, y_t = in_tiles[s]
            streams = (("x", x_t), ("y", y_t))

            # -- per-stream projection blocks: theta, phi, g --
            pj = {}
            gT = {}
            rscol = rtmp.tile([128, MC], F32, tag="rscol", name="rscol", bufs=2)
            for st, src in streams:
                for pr in ("t", "p"):
                    w = w_sb[f"wt_{pr}{st}"]
                    dst = proj.tile([128, CIC, N], FP8, tag=f"pj_{pr}{st}",
                                    name=f"pj_{pr}{st}")
                    pj[pr + st] = dst
                    for cic in range(CIC):
                        pt = ps.tile([128, N], F32, tag="ps", name="ps")
                        for k in range(KC):
                            for o, f in NH:
                                mm(pt[:, o:o + f],
                                   w[:, k, cic * 128:(cic + 1) * 128],
                                   src[:, k, o:o + f], k == 0, k == KC - 1)
                        nc.scalar.activation(
                            dst[:, cic, :], pt[:], AF.Identity,
                            bias=tb_sb[f"b_{pr}{st}"][:, cic:cic + 1])
                # g projection, (N, CI) layout; col CI carries the
                # gamma/C-scaled residual colsum of this stream
                w = w_sb[f"wt_g{st}"]
                dst = proj.tile([128, MC, CI], BF16, tag=f"gT{st}", name=f"gT{st}")
                gT[st] = dst
                for mc_ in range(MC):
                    pt = ps.tile([128, CI + 1], F32, tag="ps", name="ps")
                    has_b = st in gb_sb
                    for k in range(KC):
                        mm(pt[:], src[:, k, mc_ * 128:(mc_ + 1) * 128],
                           w[:, k, :], k == 0, (k == KC - 1) and not has_b)
                    if has_b:
                        mm(pt[:, :CI], ones_row[:], gb_sb[st][:], False, True)
                    nc.vector.tensor_copy(dst[:, mc_, :], pt[:, :CI])
                    if st == "x":
                        nc.scalar.copy(rscol[:, mc_:mc_ + 1], pt[:, CI:CI + 1])
                    else:
                        nc.vector.tensor_add(rscol[:, mc_:mc_ + 1],
                                             rscol[:, mc_:mc_ + 1],
                                             pt[:, CI:CI + 1])
                # previous sample's deferred fixup, staged so PE never
                # waits on the ACT/DVE row chain: qraw after the x-stream
                # block, transposes a full stream block later.
                if st == "x" and pending is not None:
                    pending_t = emit_fixup_qraw(pending)
                    pending = None
                elif st == "y" and pending_t is not None:
                    emit_fixup_transposes(pending_t)
                    pending_t = None

            if s + 1 < BPC:
                in_tiles.append(load_inputs(s + 1))

            # -- transposed logits + exp --
            E = {st: attp.tile([128, MC, N], BF16, tag=f"E{st}", name=f"E{st}")
                 for st, _ in streams}
            S = attp.tile([128, MC, N], BF16, tag="S", name="S")
            for mc_ in range(MC):
                for st, _ in streams:
                    pt = ps.tile([128, N], F32, tag="ps", name="ps")
                    for o, f in NH:
                        mmdr(pt[:, o:o + f],
                             pj["p" + st][:, :, mc_ * 128:(mc_ + 1) * 128],
                             pj["t" + st][:, :, o:o + f], True, True)
                    nc.scalar.activation(E[st][:, mc_, :], pt[:], AF.Exp,
                                         bias=expb[:])
                nc.vector.tensor_mul(S[:, mc_, :], E["x"][:, mc_, :],
                                     E["y"][:, mc_, :])

            # -- softmax denominators (partition sums via ones-matmul) --
            zrows = {}
            for key, st in (("zx", "x"), ("zy", "y")):
                pt = ps.tile([1, N], F32, tag="ps", name="ps")
                for mc_ in range(MC):
                    for o, f in NH:
                        mm(pt[:, o:o + f], ones_col[:], E[st][:, mc_, o:o + f],
                           mc_ == 0, mc_ == MC - 1)
                r = rtmp.tile([1, N], F32, tag=f"r_{key}", name=f"r_{key}", bufs=2)
                nc.scalar.copy(r[:], pt[:])
                zrows[key] = r

            # chain A of the fixup: R2 = 1/(Zx*Zy)^2, overlapped with U matmuls
            p1 = rtmp.tile([1, N], F32, tag="rt", name="p1")
            nc.vector.tensor_mul(p1[:], zrows["zx"][:], zrows["zy"][:])
            p2 = rtmp.tile([1, N], F32, tag="rt", name="p2")
            nc.vector.reciprocal(p2[:], p1[:])
            p3 = rtmp.tile([1, N], F32, tag="rt", name="p3")
            nc.vector.tensor_mul(p3[:], p2[:], p2[:])

            # -- unnormalized attention-apply + product --
            yv = attp.tile([128, CIC, N], BF16, tag="yv", name="yv")
            for cic in range(CIC):
                ptu = {}
                for st, _ in streams:
                    pt = ps.tile([128, N], F32, tag="ps", name="ps")
                    ptu[st] = pt
                    for mc_ in range(MC):
                        for o, f in NH:
                            mm(pt[:, o:o + f],
                               gT[st][:, mc_, cic * 128:(cic + 1) * 128],
                               S[:, mc_, o:o + f], mc_ == 0, mc_ == MC - 1)
                # DVE tensor_tensor cannot read two PSUM operands; bounce Ux
                ux_sb = rtmp.tile([128, N], BF16, tag="ux_sb", name="ux_sb", bufs=2)
                nc.scalar.copy(ux_sb[:], ptu["x"][:])
                nc.vector.tensor_mul(yv[:, cic, :], ux_sb[:], ptu["y"][:])

            pending = (s, p3, yv, wbar, rscol)

        emit_fixup_transposes(emit_fixup_qraw(pending))

        # ---- head ----
        pt = ps.tile([BPC, HOUT], F32, tag="ps", name="head_ps")
        for j in range(MC):
            mm(pt[:], pooledT[:, j, :], hwT[:, j, :],
               j == 0, (j == MC - 1) and not has_hb)
        if has_hb:
            mm(pt[:], ones_row[:, :BPC], hb[:], False, True)
        out_sb = rows.tile([BPC, HOUT], F32, tag="out_sb", name="out_sb")
        nc.scalar.copy(out_sb[:], pt[:])
        nc.sync.dma_start(d_out[:], out_sb[:])

    nc.compile()
    return nc


def _prepare(inputs):
    f = lambda k: np.ascontiguousarray(np.asarray(inputs[k], dtype=np.float32))
    bf = lambda a: np.ascontiguousarray(np.asarray(a, dtype=ml_dtypes.bfloat16))
    sar, opt = f("sar"), f("opt")
    ga = float(np.asarray(inputs["gamma_att"]).reshape(-1)[0])
    go = float(np.asarray(inputs["gamma_opt"]).reshape(-1)[0])
    gs = float(np.asarray(inputs["gamma_sar"]).reshape(-1)[0])
    W_w, W_b = f("W_w"), f("W_b")
    head_w, head_b = f("head_w"), f("head_b")

    wbar = (ga / C) * W_w.sum(axis=0)  # (CI,)
    bbar = (ga / C) * float(W_b.sum())
    # fold the pooled-constant through the head: out += bbar * head_w.sum(1)
    hb_eff = head_b + bbar * head_w.sum(axis=1)  # (HOUT,)

    gb_x, gb_y = f("g_sar_b"), f("g_opt_b")
    has_gb_x = bool(np.any(gb_x))
    has_gb_y = bool(np.any(gb_y))
    has_hb = bool(np.any(hb_eff))

    key = (has_gb_x, has_gb_y, has_hb)
    if key not in _cached:
        _cached[key] = _build(*key)
    nc = _cached[key]

    # pack inputs: (B, C, N) -> per-core (BPC, 128, KC*N) partition-major
    def pack_in(a):
        a = a.reshape(B, KC, 128, N).transpose(0, 2, 1, 3).reshape(B, 128, KC * N)
        return np.ascontiguousarray(a).astype(ml_dtypes.bfloat16)

    sar_p, opt_p = pack_in(sar), pack_in(opt)

    common = {
        "wt_tx": _pack(f("theta_sar_w").T),
        "wt_px": _pack(f("phi_sar_w").T),
        "wt_ty": _pack(f("theta_opt_w").T),
        "wt_py": _pack(f("phi_opt_w").T),
        "wt_gx": _pack(np.concatenate(
            [f("g_sar_w").T, np.full((C, 1), gs / C, np.float32)], axis=1)),
        "wt_gy": _pack(np.concatenate(
            [f("g_opt_w").T, np.full((C, 1), go / C, np.float32)], axis=1)),
        "hwT": _pack(head_w.T),
        "wbar": bf(wbar),
        "b_tx": f("theta_sar_b"), "b_px": f("phi_sar_b"),
        "b_ty": f("theta_opt_b"), "b_py": f("phi_opt_b"),
        "ones_col": np.ones((128, 1), ml_dtypes.bfloat16),
        "ident": np.eye(4, dtype=np.float32),
        "expb": np.full((128, 1), EXP_SHIFT, np.float32),
    }
    if has_gb_x or has_gb_y or has_hb:
        common["ones_row"] = np.ones((1, 128), ml_dtypes.bfloat16)
    if has_gb_x:
        common["gb_x"] = bf(gb_x.reshape(1, CI))
    if has_gb_y:
        common["gb_y"] = bf(gb_y.reshape(1, CI))
    if has_hb:
        common["hb"] = bf(hb_eff.reshape(1, HOUT))

    in_maps = []
    for c in range(NCORES):
        m = dict(common)
        m["sar"] = np.ascontiguousarray(sar_p[c * BPC:(c + 1) * BPC])
        m["opt"] = np.ascontiguousarray(opt_p[c * BPC:(c + 1) * BPC])
        in_maps.append(m)
    return nc, in_maps


def kernel(**inputs):
    nc, in_maps = _prepare(inputs)
    res = run_bass_kernel_spmd(nc, in_maps, core_ids=list(range(NCORES)))
    return np.concatenate([res.results[c]["out"] for c in range(NCORES)], axis=0)


if __name__ == "__main__":
    rng = np.random.default_rng(0)
    ins = {
        "sar": rng.standard_normal((B, C, N), dtype=np.float32),
        "opt": rng.standard_normal((B, C, N), dtype=np.float32),
    }
    for nm in ("g_sar", "g_opt", "theta_sar", "theta_opt", "phi_sar", "phi_opt"):
        ins[nm + "_w"] = 0.02 * rng.standard_normal((CI, C), dtype=np.float32)
        ins[nm + "_b"] = np.zeros((CI,), np.float32)
    ins["W_w"] = 0.02 * rng.standard_normal((C, CI), dtype=np.float32)
    ins["W_b"] = np.zeros((C,), np.float32)
    ins["head_w"] = 0.02 * rng.standard_normal((HOUT, N), dtype=np.float32)
    ins["head_b"] = np.zeros((HOUT,), np.float32)
    ins["gamma_sar"] = np.asarray([0.3], np.float32)
    ins["gamma_opt"] = np.asarray([1.0], np.float32)
    ins["gamma_att"] = np.asarray([1.0], np.float32)
    out = kernel(**ins)
    print(out.shape, out.dtype, np.abs(out).mean())


# revision 23
# speedup vs baseline: 1.1440x; 1.0717x over previous
"""Trainium2 Bass kernel for nn_CAFF_3100966388292.

Dual-stream (SAR/OPT) cross-attention fusion net:
  theta/phi/g 1x1-conv projections on both streams, per-sample NxN attention
  maps fused elementwise, both value streams attended, product taken, output
  1x1-conv + residual + channel-mean pool + linear head.

Strategy (pure data parallel, 4 samples per core on 8 cores):
  * Layouts chosen so no on-device transposes of big tensors are needed:
      - theta/phi in (CI, N) layout  (lhsT = host-pretransposed weights)
      - g directly in (N, CI) layout (lhsT = input tile, rhs = w^T)
      - attention logits computed TRANSPOSED: L^T(m,n) = phi^T theta, so the
        contracted dim (m) of att@g lands on PSUM partitions naturally.
  * Softmax denominators via ones-column matmuls on the tensor engine
    (partition-dim sums), applied as a scalar fixup on the pooled row:
      (att@g_x * att@g_y)(n,:) = Ux(:,n)*Uy(:,n) / (Zx(n)*Zy(n))^2
    with U the unnormalized attended values (global EXP_SHIFT cancels too).
  * The final W-projection + residual + channel-mean + head collapse
    algebraically:
      pooled(n) = R2(n)*qraw(n) + (ga/C)*sum(W_b) + rs(n),
      rs(n)     = (go/C)*colsum(opt)(n) + (gs/C)*colsum(sar)(n)  [one PSUM acc]
      qraw(n)   = sum_ci wbar(ci) * Ux(ci,n) * Uy(ci,n),
      wbar      = (ga/C) * W_w.sum(0)
    which removes the (C,CI)x(CI,N) W matmul entirely.
  * bf16 on all matmul operands (host-casts + host-packs inputs/weights into
    partition-major layout for single large contiguous-line DMAs). PSUM and
    the pooled fixup chain stay fp32. Final rel err ~3e-3, dominated by bf16
    rounding of the residual colsum path.
  * The per-sample fixup chain + pooled-row transposes are emitted deferred
    (inside the next sample's projection phase) so the PE never stalls on the
    serial DVE row chain.
"""

import sys
import types

import ml_dtypes
import numpy as np

# The agent image's antenv package lacks axon_hooks; register the equivalent
# NTFF hook so run_bass_kernel_spmd(trace=True) works if ever requested.
try:  # pragma: no cover
    import antenv.axon_hooks  # noqa: F401
except ImportError:
    try:
        from trn_agent_boot.trn_boot import _ntff_profile_via_ctypes

        _hook = _ntff_profile_via_ctypes("/opt/axon/libaxon_pjrt.so")
        _mod = types.ModuleType("antenv.axon_hooks")
        _mod.get_axon_ntff_profile_hook = lambda: _hook
        _mod.set_axon_ntff_profile_hook = lambda h: None
        sys.modules["antenv.axon_hooks"] = _mod
    except Exception:
        pass

import concourse.bass as bass
import concourse.tile as tile
from concourse import bacc, mybir
from concourse.bass_utils import run_bass_kernel_spmd

F32 = mybir.dt.float32
BF16 = mybir.dt.bfloat16
FP8 = mybir.dt.float8e4
EXP_SHIFT = -12.0  # constant logit shift before exp; cancels exactly in the math

B, C, CI, N, HOUT = 32, 512, 256, 768, 256
NCORES = 8
BPC = B // NCORES  # samples per core
KC = C // 128  # 4 k-chunks over channels
MC = N // 128  # 6 chunks over positions
CIC = CI // 128  # 2 chunks over inner channels
# free-dim split of N into PSUM-bank-legal matmul halves
NH = ((0, 512), (512, 256))

_cached = {}


def _pack(a):
    """(R, F) host array -> (128, R//128 * F) partition-major bf16."""
    a = np.asarray(a, dtype=np.float32)
    r, f = a.shape
    k = r // 128
    return np.ascontiguousarray(
        a.reshape(k, 128, f).transpose(1, 0, 2).reshape(128, k * f)
    ).astype(ml_dtypes.bfloat16)


def _build(has_gb_x, has_gb_y, has_hb):
    nc = bacc.Bacc("TRN2", target_bir_lowering=False, debug=False)
    AF = mybir.ActivationFunctionType

    def mm(out, lhsT, rhs, start, stop):
        nc.tensor.matmul(out, lhsT, rhs, start=start, stop=stop)

    def mmdr(out, lhsT, rhs, start, stop):
        nc.tensor.matmul(out, lhsT, rhs, start=start, stop=stop,
                         perf_mode=mybir.MatmulPerfMode.DoubleRow)

    # inputs host-packed to (BPC, 128, KC*N) partition-major bf16
    d_sar = nc.dram_tensor("sar", [BPC, 128, KC * N], BF16, kind="ExternalInput")
    d_opt = nc.dram_tensor("opt", [BPC, 128, KC * N], BF16, kind="ExternalInput")
    d_sar8 = nc.dram_tensor("sar8", [BPC, 128, KC * N], FP8, kind="ExternalInput")
    d_opt8 = nc.dram_tensor("opt8", [BPC, 128, KC * N], FP8, kind="ExternalInput")
    # host-pretransposed + packed projection weights, (128, KC*CI) bf16
    d_w = {}
    for nm in ("wt_tx", "wt_px", "wt_ty", "wt_py"):
        d_w[nm] = nc.dram_tensor(nm, [128, KC * CI], FP8, kind="ExternalInput")
    for nm in ("wt_gx", "wt_gy"):  # g weights carry a gamma/C ones column
        d_w[nm] = nc.dram_tensor(nm, [128, KC * (CI + 1)], BF16,
                                 kind="ExternalInput")
    d_hwT = nc.dram_tensor("hwT", [128, MC * HOUT], BF16, kind="ExternalInput")
    d_wbar = nc.dram_tensor("wbar", [CI], BF16, kind="ExternalInput")
    d_tb = {  # theta/phi per-partition bias columns (CI,), fp32 (ACT bias)
        nm: nc.dram_tensor(nm, [CI], F32, kind="ExternalInput")
        for nm in ("b_tx", "b_px", "b_ty", "b_py")
    }
    d_ones = nc.dram_tensor("ones_col", [128, 1], BF16, kind="ExternalInput")
    need_onesr = has_gb_x or has_gb_y or has_hb
    if need_onesr:
        d_onesr = nc.dram_tensor("ones_row", [1, 128], BF16, kind="ExternalInput")
    d_ident = nc.dram_tensor("ident", [4, 4], F32, kind="ExternalInput")
    d_expb = nc.dram_tensor("expb", [128, 1], F32, kind="ExternalInput")
    d_gb = {}
    if has_gb_x:
        d_gb["x"] = nc.dram_tensor("gb_x", [1, CI], BF16, kind="ExternalInput")
    if has_gb_y:
        d_gb["y"] = nc.dram_tensor("gb_y", [1, CI], BF16, kind="ExternalInput")
    if has_hb:
        d_hb = nc.dram_tensor("hb", [1, HOUT], BF16, kind="ExternalInput")
    d_out = nc.dram_tensor("out", [BPC, HOUT], F32, kind="ExternalOutput")

    with tile.TileContext(nc) as tc, \
            tc.tile_pool(name="wts", bufs=1) as wts, \
            tc.tile_pool(name="inp", bufs=2) as inp, \
            tc.tile_pool(name="proj", bufs=1) as proj, \
            tc.tile_pool(name="att", bufs=1) as attp, \
            tc.tile_pool(name="rows", bufs=1) as rows, \
            tc.tile_pool(name="rtmp", bufs=4) as rtmp, \
            tc.tile_pool(name="ps", bufs=4, space="PSUM") as ps:

        # ---- DMAs in strict first-use order: the queues are FIFO, so
        # everything emitted ahead of the first matmul's dependencies delays
        # kernel start ----
        def load_w(nm, cols=CI, dt_=None):
            t = wts.tile([128, KC, cols],
                         dt_ or (FP8 if nm[3] in "tp" else BF16), tag=nm, name=nm)
            nc.sync.dma_start(t[:], d_w[nm].ap().rearrange("p (k f) -> p k f", k=KC))
            return t

        w_sb = {}
        # first weight + first input chunk gate the whole kernel: issue the
        # k0 pieces first so matmul #1 waits on ~256KB, not ~1MB
        t = wts.tile([128, KC, CI], FP8, tag="wt_tx", name="wt_tx")
        w_sb["wt_tx"] = t
        nc.sync.dma_start(t[:, 0, :], d_w["wt_tx"].ap()[:, :CI])
        x8_0 = inp.tile([128, KC, N], FP8, tag="x8", name="x8")
        nc.sync.dma_start(x8_0[:, 0:2, :],
                          d_sar8[0][:, :2 * N].rearrange("p (k n) -> p k n", k=2))
        nc.sync.dma_start(
            t[:, 1:, :],
            d_w["wt_tx"].ap()[:, CI:].rearrange("p (k f) -> p k f", k=KC - 1))
        nc.sync.dma_start(x8_0[:, 2:, :],
                          d_sar8[0][:, 2 * N:].rearrange("p (k n) -> p k n", k=2))
        w_sb["wt_px"] = load_w("wt_px")
        x0 = inp.tile([128, KC, N], BF16, tag="x_t", name="x_t")
        for k in range(KC):
            nc.sync.dma_start(x0[:, k, :], d_sar[0][:, k * N:(k + 1) * N])
        w_sb["wt_gx"] = load_w("wt_gx", CI + 1)
        tb_sb = {}
        for nm, d in d_tb.items():
            t = wts.tile([128, CIC], F32, tag=nm, name=nm)
            nc.sync.dma_start(t[:], d.ap().rearrange("(k p) -> p k", p=128))
            tb_sb[nm] = t
        w_sb["wt_ty"] = load_w("wt_ty")
        w_sb["wt_py"] = load_w("wt_py")
        w_sb["wt_gy"] = load_w("wt_gy", CI + 1)
        y8_0 = inp.tile([128, KC, N], FP8, tag="y8", name="y8")
        nc.sync.dma_start(y8_0[:], d_opt8[0].rearrange("p (k n) -> p k n", k=KC))
        y0 = inp.tile([128, KC, N], BF16, tag="y_t", name="y_t")
        for k in range(KC):
            nc.sync.dma_start(y0[:, k, :], d_opt[0][:, k * N:(k + 1) * N])

        def load_inputs(s):
            x_t = inp.tile([128, KC, N], BF16, tag="x_t", name="x_t")
            y_t = inp.tile([128, KC, N], BF16, tag="y_t", name="y_t")
            x8 = inp.tile([128, KC, N], FP8, tag="x8", name="x8")
            y8 = inp.tile([128, KC, N], FP8, tag="y8", name="y8")
            nc.sync.dma_start(x8[:], d_sar8[s].rearrange("p (k n) -> p k n", k=KC))
            nc.sync.dma_start(y8[:], d_opt8[s].rearrange("p (k n) -> p k n", k=KC))
            for k in range(KC):
                nc.sync.dma_start(x_t[:, k, :], d_sar[s][:, k * N:(k + 1) * N])
            for k in range(KC):
                nc.sync.dma_start(y_t[:, k, :], d_opt[s][:, k * N:(k + 1) * N])
            return x_t, y_t, x8, y8

        in_tiles = [(x0, y0, x8_0, y8_0)]

        # ---- small constants (all needed later than the projections) ----
        wbar = wts.tile([128, CIC], BF16, tag="wbar", name="wbar")
        nc.sync.dma_start(wbar[:], d_wbar.ap().rearrange("(k p) -> p k", p=128))
        ones_col = wts.tile([128, 1], BF16, tag="ones_col", name="ones_col")
        nc.sync.dma_start(ones_col[:], d_ones.ap())
        ident = wts.tile([4, 4], F32, tag="ident", name="ident")
        nc.sync.dma_start(ident[:], d_ident.ap())
        expb = wts.tile([128, 1], F32, tag="expb", name="expb")
        nc.sync.dma_start(expb[:], d_expb.ap())
        hwT = wts.tile([128, MC, HOUT], BF16, tag="hwT", name="hwT")
        nc.sync.dma_start(hwT[:], d_hwT.ap().rearrange("p (k f) -> p k f", k=MC))
        if need_onesr:
            ones_row = wts.tile([1, 128], BF16, tag="ones_row", name="ones_row")
            nc.sync.dma_start(ones_row[:], d_onesr.ap())
        gb_sb = {}
        for st, d in d_gb.items():
            t = wts.tile([1, CI], BF16, tag=f"gb_{st}", name=f"gb_{st}")
            nc.sync.dma_start(t[:], d.ap())
            gb_sb[st] = t
        if has_hb:
            hb = wts.tile([1, HOUT], BF16, tag="hb", name="hb")
            nc.sync.dma_start(hb[:], d_hb.ap())

        pooledT = rows.tile([128, MC, BPC], BF16, tag="pooledT", name="pooledT")

        def emit_fixup_qraw(fx):
            """qraw matvec + chain B (PE then ACT/DVE latency off PE path)."""
            s, p3, yv, wbar_, rscol = fx
            pt = ps.tile([1, N], F32, tag="ps", name="ps")
            for cic in range(CIC):
                for o, f in NH:
                    mm(pt[:, o:o + f], wbar_[:, cic:cic + 1],
                       yv[:, cic, o:o + f], cic == 0, cic == CIC - 1)
            q_row = rtmp.tile([1, N], F32, tag="r_q", name="q_row", bufs=2)
            nc.scalar.copy(q_row[:], pt[:])
            p4 = rtmp.tile([1, N], F32, tag="rt", name="p4")
            nc.vector.tensor_mul(p4[:], p3[:], q_row[:])
            return (s, p4, rscol)

        def emit_fixup_transposes(fx):
            s, p4, rscol = fx
            for j in range(MC):
                tp_ = ps.tile([128, 1], F32, tag="ps", name="tp_")
                nc.tensor.transpose(tp_[:],
                                    p4[:, j * 128:(j + 1) * 128],
                                    ident[:1, :1])
                nc.vector.tensor_add(pooledT[:, j, s:s + 1], tp_[:],
                                     rscol[:, j:j + 1])

        pending = None
        pending_t = None
        for s in range(BPC):
            x_t, y_t, x8, y8 = in_tiles[s]
            streams = (("x", x_t), ("y", y_t))
            s8 = {"x": x8, "y": y8}

            # -- per-stream projection blocks: theta, phi, g --
            pj = {}
            gT = {}
            rscol = rtmp.tile([128, MC], F32, tag="rscol", name="rscol", bufs=2)
            for st, src in streams:
                for pr in ("t", "p"):
                    w = w_sb[f"wt_{pr}{st}"]
                    dst = proj.tile([128, CIC, N], FP8, tag=f"pj_{pr}{st}",
                                    name=f"pj_{pr}{st}")
                    pj[pr + st] = dst
                    for cic in range(CIC):
                        pt = ps.tile([128, N], F32, tag="ps", name="ps")
                        for kp in range(KC // 2):
                            for o, f in NH:
                                mmdr(pt[:, o:o + f],
                                     w[:, 2 * kp:2 * kp + 2,
                                       cic * 128:(cic + 1) * 128],
                                     s8[st][:, 2 * kp:2 * kp + 2, o:o + f],
                                     kp == 0, kp == KC // 2 - 1)
                        nc.scalar.activation(
                            dst[:, cic, :], pt[:], AF.Identity,
                            bias=tb_sb[f"b_{pr}{st}"][:, cic:cic + 1])
                # g projection, (N, CI) layout; col CI carries the
                # gamma/C-scaled residual colsum of this stream
                w = w_sb[f"wt_g{st}"]
                dst = proj.tile([128, MC, CI], BF16, tag=f"gT{st}", name=f"gT{st}")
                gT[st] = dst
                for mc_ in range(MC):
                    pt = ps.tile([128, CI + 1], F32, tag="ps", name="ps")
                    has_b = st in gb_sb
                    for k in range(KC):
                        mm(pt[:], src[:, k, mc_ * 128:(mc_ + 1) * 128],
                           w[:, k, :], k == 0, (k == KC - 1) and not has_b)
                    if has_b:
                        mm(pt[:, :CI], ones_row[:], gb_sb[st][:], False, True)
                    nc.vector.tensor_copy(dst[:, mc_, :], pt[:, :CI])
                    if st == "x":
                        nc.scalar.copy(rscol[:, mc_:mc_ + 1], pt[:, CI:CI + 1])
                    else:
                        nc.vector.tensor_add(rscol[:, mc_:mc_ + 1],
                                             rscol[:, mc_:mc_ + 1],
                                             pt[:, CI:CI + 1])
                # previous sample's deferred fixup, staged so PE never
                # waits on the ACT/DVE row chain: qraw after the x-stream
                # block, transposes a full stream block later.
                if st == "x" and pending is not None:
                    pending_t = emit_fixup_qraw(pending)
                    pending = None
                elif st == "y" and pending_t is not None:
                    emit_fixup_transposes(pending_t)
                    pending_t = None

            if s + 1 < BPC:
                in_tiles.append(load_inputs(s + 1))

            # -- transposed logits + exp --
            E = {st: attp.tile([128, MC, N], BF16, tag=f"E{st}", name=f"E{st}")
                 for st, _ in streams}
            S = attp.tile([128, MC, N], BF16, tag="S", name="S")
            for mc_ in range(MC):
                for st, _ in streams:
                    pt = ps.tile([128, N], F32, tag="ps", name="ps")
                    for o, f in NH:
                        mmdr(pt[:, o:o + f],
                             pj["p" + st][:, :, mc_ * 128:(mc_ + 1) * 128],
                             pj["t" + st][:, :, o:o + f], True, True)
                    nc.scalar.activation(E[st][:, mc_, :], pt[:], AF.Exp,
                                         bias=expb[:])
                nc.vector.tensor_mul(S[:, mc_, :], E["x"][:, mc_, :],
                                     E["y"][:, mc_, :])

            # -- softmax denominators (partition sums via ones-matmul) --
            zrows = {}
            for key, st in (("zx", "x"), ("zy", "y")):
                pt = ps.tile([1, N], F32, tag="ps", name="ps")
                for mc_ in range(MC):
                    for o, f in NH:
                        mm(pt[:, o:o + f], ones_col[:], E[st][:, mc_, o:o + f],
                           mc_ == 0, mc_ == MC - 1)
                r = rtmp.tile([1, N], F32, tag=f"r_{key}", name=f"r_{key}", bufs=2)
                nc.scalar.copy(r[:], pt[:])
                zrows[key] = r

            # chain A of the fixup: R2 = 1/(Zx*Zy)^2, overlapped with U matmuls
            p1 = rtmp.tile([1, N], F32, tag="rt", name="p1")
            nc.vector.tensor_mul(p1[:], zrows["zx"][:], zrows["zy"][:])
            p2 = rtmp.tile([1, N], F32, tag="rt", name="p2")
            nc.vector.reciprocal(p2[:], p1[:])
            p3 = rtmp.tile([1, N], F32, tag="rt", name="p3")
            nc.vector.tensor_mul(p3[:], p2[:], p2[:])

            # -- unnormalized attention-apply + product --
            yv = attp.tile([128, CIC, N], BF16, tag="yv", name="yv")
            for cic in range(CIC):
                ptu = {}
                for st, _ in streams:
                    pt = ps.tile([128, N], F32, tag="ps", name="ps")
                    ptu[st] = pt
                    for mc_ in range(MC):
                        for o, f in NH:
                            mm(pt[:, o:o + f],
                               gT[st][:, mc_, cic * 128:(cic + 1) * 128],
                               S[:, mc_, o:o + f], mc_ == 0, mc_ == MC - 1)
                # DVE tensor_tensor cannot read two PSUM operands; bounce Ux
                ux_sb = rtmp.tile([128, N], BF16, tag="ux_sb", name="ux_sb", bufs=2)
                nc.scalar.copy(ux_sb[:], ptu["x"][:])
                nc.vector.tensor_mul(yv[:, cic, :], ux_sb[:], ptu["y"][:])

            pending = (s, p3, yv, wbar, rscol)

        emit_fixup_transposes(emit_fixup_qraw(pending))

        # ---- head ----
        pt = ps.tile([BPC, HOUT], F32, tag="ps", name="head_ps")
        for j in range(MC):
            mm(pt[:], pooledT[:, j, :], hwT[:, j, :],
               j == 0, (j == MC - 1) and not has_hb)
        if has_hb:
            mm(pt[:], ones_row[:, :BPC], hb[:], False, True)
        out_sb = rows.tile([BPC, HOUT], F32, tag="out_sb", name="out_sb")
        nc.scalar.copy(out_sb[:], pt[:])
        nc.sync.dma_start(d_out[:], out_sb[:])

    nc.compile()
    return nc


def _prepare(inputs):
    f = lambda k: np.ascontiguousarray(np.asarray(inputs[k], dtype=np.float32))
    bf = lambda a: np.ascontiguousarray(np.asarray(a, dtype=ml_dtypes.bfloat16))
    sar, opt = f("sar"), f("opt")
    ga = float(np.asarray(inputs["gamma_att"]).reshape(-1)[0])
    go = float(np.asarray(inputs["gamma_opt"]).reshape(-1)[0])
    gs = float(np.asarray(inputs["gamma_sar"]).reshape(-1)[0])
    W_w, W_b = f("W_w"), f("W_b")
    head_w, head_b = f("head_w"), f("head_b")

    wbar = (ga / C) * W_w.sum(axis=0)  # (CI,)
    bbar = (ga / C) * float(W_b.sum())
    # fold the pooled-constant through the head: out += bbar * head_w.sum(1)
    hb_eff = head_b + bbar * head_w.sum(axis=1)  # (HOUT,)

    gb_x, gb_y = f("g_sar_b"), f("g_opt_b")
    has_gb_x = bool(np.any(gb_x))
    has_gb_y = bool(np.any(gb_y))
    has_hb = bool(np.any(hb_eff))

    key = (has_gb_x, has_gb_y, has_hb)
    if key not in _cached:
        _cached[key] = _build(*key)
    nc = _cached[key]

    # pack inputs: (B, C, N) -> per-core (BPC, 128, KC*N) partition-major
    def pack_in(a):
        a = a.reshape(B, KC, 128, N).transpose(0, 2, 1, 3).reshape(B, 128, KC * N)
        return np.ascontiguousarray(a).astype(ml_dtypes.bfloat16)

    sar_p, opt_p = pack_in(sar), pack_in(opt)

    p8 = lambda a: _pack(a).astype(ml_dtypes.float8_e4m3fn)
    common = {
        "wt_tx": p8(f("theta_sar_w").T),
        "wt_px": p8(f("phi_sar_w").T),
        "wt_ty": p8(f("theta_opt_w").T),
        "wt_py": p8(f("phi_opt_w").T),
        "wt_gx": _pack(np.concatenate(
            [f("g_sar_w").T, np.full((C, 1), gs / C, np.float32)], axis=1)),
        "wt_gy": _pack(np.concatenate(
            [f("g_opt_w").T, np.full((C, 1), go / C, np.float32)], axis=1)),
        "hwT": _pack(head_w.T),
        "wbar": bf(wbar),
        "b_tx": f("theta_sar_b"), "b_px": f("phi_sar_b"),
        "b_ty": f("theta_opt_b"), "b_py": f("phi_opt_b"),
        "ones_col": np.ones((128, 1), ml_dtypes.bfloat16),
        "ident": np.eye(4, dtype=np.float32),
        "expb": np.full((128, 1), EXP_SHIFT, np.float32),
    }
    if has_gb_x or has_gb_y or has_hb:
        common["ones_row"] = np.ones((1, 128), ml_dtypes.bfloat16)
    if has_gb_x:
        common["gb_x"] = bf(gb_x.reshape(1, CI))
    if has_gb_y:
        common["gb_y"] = bf(gb_y.reshape(1, CI))
    if has_hb:
        common["hb"] = bf(hb_eff.reshape(1, HOUT))

    in_maps = []
    for c in range(NCORES):
        m = dict(common)
        m["sar"] = np.ascontiguousarray(sar_p[c * BPC:(c + 1) * BPC])
        m["opt"] = np.ascontiguousarray(opt_p[c * BPC:(c + 1) * BPC])
        m["sar8"] = m["sar"].astype(ml_dtypes.float8_e4m3fn)
        m["opt8"] = m["opt"].astype(ml_dtypes.float8_e4m3fn)
        in_maps.append(m)
    return nc, in_maps


def kernel(**inputs):
    nc, in_maps = _prepare(inputs)
    res = run_bass_kernel_spmd(nc, in_maps, core_ids=list(range(NCORES)))
    return np.concatenate([res.results[c]["out"] for c in range(NCORES)], axis=0)


if __name__ == "__main__":
    rng = np.random.default_rng(0)
    ins = {
        "sar": rng.standard_normal((B, C, N), dtype=np.float32),
        "opt": rng.standard_normal((B, C, N), dtype=np.float32),
    }
    for nm in ("g_sar", "g_opt", "theta_sar", "theta_opt", "phi_sar", "phi_opt"):
        ins[nm + "_w"] = 0.02 * rng.standard_normal((CI, C), dtype=np.float32)
        ins[nm + "_b"] = np.zeros((CI,), np.float32)
    ins["W_w"] = 0.02 * rng.standard_normal((C, CI), dtype=np.float32)
    ins["W_b"] = np.zeros((C,), np.float32)
    ins["head_w"] = 0.02 * rng.standard_normal((HOUT, N), dtype=np.float32)
    ins["head_b"] = np.zeros((HOUT,), np.float32)
    ins["gamma_sar"] = np.asarray([0.3], np.float32)
    ins["gamma_opt"] = np.asarray([1.0], np.float32)
    ins["gamma_att"] = np.asarray([1.0], np.float32)
    out = kernel(**ins)
    print(out.shape, out.dtype, np.abs(out).mean())
